# revision 19
# baseline (speedup 1.0000x reference)
"""Trainium2 Bass kernel for bilinear forward-warp splatting (scatter_memory).

Computes, per batch element b (data-parallel over 8 NeuronCores):
    wy = y0 + dt*fy;  wx = x0 + dt*fx          (dt = tref - i)
    out[y, x] = sum_p v_p * tent(wy_p - y) * tent(wx_p - x)
for the three channels v in {1, fy, fx}, where tent(u) = max(0, 1-|u|) is
exactly the bilinear splat weight, followed by wf = splat(w*f)/(splat(w)+eps).

Algorithm: displacement dt*flow is bounded (verified on host per region), so
the scatter is a windowed rank-1 accumulation: for each chunk of 128 points
(32 rows x 4 columns of the grid), build a sparse "tent" matrix over the
y-window (lhsT [128, mY]) and the x-window x 3 channels (rhs) with a single
fused custom-DVE op (relu(1-|iota-w|)), then let the TensorEngine accumulate
sum_p tentY[p,:]^T (x) rhs[p,:] into PSUM, sliding along 32-row bands.
PSUM segments are converted to bf16 on the ACT engine and added (2x DVE mode)
into bf16 SBUF grid accumulators.  Windows are per-(band,segment): the y
bottom and both x sides are trimmed to the regional displacement maxima.
W=640 splits into 5 uniform PSUM segments of 32 column-groups (psum sized by
the regional window maxima, not the global bound).  Prep (warp coords,
in-bounds mask, interleave split into band point order) runs per 128-row
block right before its bands with contiguous DVE writes + GpSimd strided
reads; per segment, the (fy, fx) channel pair is broadcast-expanded in ONE
ACT pass and both rhs channel muls run as ONE fused 2x-mode DVE op.
Normalization + output DMA run per 128-row storage block as soon as its last
contributing band has been spilled, overlapping the main loop.

Perf notes (traced on HW): the DVE is the saturated engine (~93%); the tent
ops are f32-in so they run at 1x (fp16 coords would be 2x but coordinate
rounding near integer boundaries flips lone tiny-weight pixels => rel err
0.7, measured); GpSimd elementwise is 6-14 cyc/elem AND its SBUF traffic
slows 2-port DVE ops ~46% (shared port), so offloading DVE work there is a
net loss; strided (stride-16B) SBUF-to-SBUF DMA runs at element rate
(~6.4us per 20KB), so band loads must stage through the GpSimd interleave
copies + contiguous DMA.
"""

import os
import sys
import math

import numpy as np

for _p in ("/opt/trn_rl_repo", "/root/.axon_site/_ro/trn_rl_repo"):
    if os.path.isdir(_p) and _p not in sys.path:
        sys.path.insert(0, _p)

from contextlib import ExitStack

import concourse.bass as bass
import concourse.bacc as bacc
import concourse.tile as tile
from concourse import mybir
from concourse.ap import AP
from concourse.bass_utils import run_bass_kernel_spmd

H, W = 480, 640
NCORES = 8
F32 = mybir.dt.float32
BF16 = mybir.dt.bfloat16  # bf16: f32 exponent range (tent weights down to ~1e-9 must not flush to 0)
Alu = mybir.AluOpType
Act = mybir.ActivationFunctionType

BH = 32          # band height (rows per band)
IL = 4           # column interleave; chunks are BH rows x IL cols = 128 points
EPS = 1e-9
BIG = 4.0e6      # pushed onto wy for masked-out points -> tent == 0 everywhere

# engine assignment knobs (A/B-tunable): which engine runs the fused rhs
# channel muls, the spill grid-accumulate add, and the prep mask chain
MUL_ON_GPSIMD = False
ADD_ON_GPSIMD = False
PREP_ON_GPSIMD = False

_TENT_OP = None


def _tent_op():
    """Register (once) the fused tent op: out = relu(1 - |in0 - in1|)."""
    global _TENT_OP
    if _TENT_OP is not None:
        return _TENT_OP
    from concourse import dve_ops as dvo
    from concourse.dve_spec import Spec, Src0, Src1, One, maxx, relu, lower
    from concourse.dve_uop import DveOpSpec

    name = "TENT_ANT"
    for op in dvo.OPS:
        if op.name == name:
            _TENT_OP = op
            return op
    spec = Spec(
        body=relu(One - maxx(Src0 - Src1, Src1 - Src0)),
        reference=lambda in0, in1, s0, s1, imm2: np.maximum(
            0.0, 1.0 - np.abs(in0 - in1)
        ),
    )
    row = dvo._CUSTOM_DVE_ROW_BASE + len(dvo.OPS)
    shas = {}
    for ver in ("v3", "v4"):
        shas[ver] = DveOpSpec(
            name=name, opcode=row, uops=lower(spec, ver=ver), rd1_en=True
        ).sha(ver)
    op = dvo.DveOp(name, spec, subdim=False, uops_sha=shas)
    dvo.OPS.append(op)
    dvo._SUB_OPCODE_FOR_NAME[name] = row
    dvo.CUSTOM_DVE_SPECS[name] = spec
    _TENT_OP = op
    return op


def _v(ap, dims, extra_off=0, parts=None):
    """Manual AP view: keep ap's partition pair, replace free dims."""
    ppair = [ap.ap[0][0], ap.ap[0][1] if parts is None else parts]
    return AP(tensor=ap.tensor, offset=ap.offset + extra_off, ap=[ppair] + [list(d) for d in dims])


def _build_program(disp, dt, dyp_map, dxn_map, dxp_map, SP, XT3, H=H, W=W):
    """disp: global y half-window (top slack, fixed for 32-aligned spills);
    dyp_map/dxn_map/dxp_map[band][seg]: per-region bottom-y / left-x / right-x
    displacement bounds; SP: column-groups (of IL columns) per PSUM segment;
    XT3: psum free extent (max over regions, <= 512 f32)."""
    TENT = _tent_op()
    PAD = disp + 1
    dypmax = max(max(r) for r in dyp_map)
    dxnmax = max(max(r) for r in dxn_map)
    dxpmax = max(max(r) for r in dxp_map)
    YWMAX = PAD + BH + dypmax + 1           # max y-window of a band
    bands = [(a, min(BH, H - a)) for a in range(0, H, BH)]
    npairs = W // IL
    nsegs = (npairs + SP - 1) // SP
    XWMAX = dxnmax + dxpmax + 2 + IL
    XT3MAX = XT3                            # psum extent (ch-inner) of a segment
    assert XT3MAX <= 512
    NBLK = (H + 127) // 128                 # 128-row blocks for plane storage
    # Grid accumulator rows are stored SHIFTED by +PAD ("storage row" =
    # real row + PAD) so every PSUM spill lands at a 32-aligned partition
    # start (engines require 32-aligned start partitions). Rows [0, PAD) and
    # [H+PAD, ...) of storage only ever accumulate exact zeros.
    assert PAD <= 32, "grid storage shift assumes PAD <= 32"
    NSBLK = (H + PAD + 127) // 128          # storage blocks for the grid
    for m in (dyp_map, dxn_map, dxp_map):
        assert len(m) == len(bands) and all(len(r) == nsegs for r in m)

    nc = bacc.Bacc("TRN2", target_bir_lowering=False, debug=False)
    fy_in = nc.declare_dram_parameter("fy", [H, W], F32, isOutput=False)
    fx_in = nc.declare_dram_parameter("fx", [H, W], F32, isOutput=False)
    o_wfx = nc.declare_dram_parameter("out_wfx", [H, W], F32, isOutput=True)
    o_wfy = nc.declare_dram_parameter("out_wfy", [H, W], F32, isOutput=True)

    mul_eng = None
    add_eng = None

    with ExitStack() as ctx:
        tc = ctx.enter_context(tile.TileContext(nc))
        mul_eng = nc.gpsimd if MUL_ON_GPSIMD else nc.vector
        add_eng = nc.gpsimd if ADD_ON_GPSIMD else nc.vector
        singles = ctx.enter_context(tc.tile_pool(name="singles", bufs=1))

        # ---- constant ramps (f32 iotas: all values exact below 2^24) ----
        NY = H - BH + YWMAX + 8
        NX = W + XWMAX + 8  # covers all slices
        ioY = singles.tile([128, NY], F32)
        ioX = singles.tile([128, NX], F32)
        x0f = singles.tile([128, W], F32)
        y0f = singles.tile([128, NBLK], F32)
        nc.gpsimd.iota(ioY[:], pattern=[[1, NY]], base=-PAD, channel_multiplier=0,
                       allow_small_or_imprecise_dtypes=True)
        nc.gpsimd.iota(ioX[:], pattern=[[1, NX]], base=-(dxnmax + 1), channel_multiplier=0,
                       allow_small_or_imprecise_dtypes=True)
        nc.gpsimd.iota(x0f[:], pattern=[[1, W]], base=0, channel_multiplier=0,
                       allow_small_or_imprecise_dtypes=True)
        nc.gpsimd.iota(y0f[:], pattern=[[128, NBLK]], base=0, channel_multiplier=1,
                       allow_small_or_imprecise_dtypes=True)

        # ---- grid accumulators (ch-inner: 0=w, 1=w*fy, 2=w*fx), +PAD shift ----
        grid = singles.tile([128, NSBLK, W, 3], BF16)
        nc.vector.memset(grid[:, :, :W // 2], 0.0)
        nc.gpsimd.memset(grid[:, :, W // 2:], 0.0)

        # zero operands for the per-segment PSUM-clearing matmul
        z_l = singles.tile([16, YWMAX], BF16)
        z_r = singles.tile([16, 512], BF16)
        epsb = singles.tile([128, 1], F32)
        nc.gpsimd.memset(z_l[:], 0.0)
        nc.gpsimd.memset(z_r[:], 0.0)
        nc.gpsimd.memset(epsb[:], EPS)

        # ---- inputs (all blocks up front; per-block prep happens in-loop) ----
        # in_f plane 0 = fy, plane 1 = fx
        in_f = singles.tile([128, 2, NBLK, W], F32)
        for blk in range(NBLK):
            rows = min(128, H - 128 * blk)
            rsl = slice(128 * blk, 128 * blk + rows)
            # column-split each plane across both DMA paths so the first
            # block's load (which gates the whole prep chain) lands ~2x faster
            nc.sync.dma_start(out=in_f[:rows, 0, blk, :W // 2],
                              in_=_v(fy_in.ap()[rsl], [[1, W // 2]]))
            nc.scalar.dma_start(out=in_f[:rows, 0, blk, W // 2:],
                                in_=_v(fy_in.ap()[rsl], [[1, W // 2]], extra_off=W // 2))
            nc.scalar.dma_start(out=in_f[:rows, 1, blk, :W // 2],
                                in_=_v(fx_in.ap()[rsl], [[1, W // 2]]))
            nc.sync.dma_start(out=in_f[:rows, 1, blk, W // 2:],
                              in_=_v(fx_in.ap()[rsl], [[1, W // 2]], extra_off=W // 2))

        # split-layout planes the band loads DMA from:
        # PSc: plane 0 = wyM (masked warped y), plane 1 = wx   (f32)
        # PSv: plane 0 = fy, plane 1 = fx                      (bf16)
        PSc = singles.tile([128, 2, NBLK, IL, W // IL], F32)
        PSv = singles.tile([128, 2, NBLK, IL, W // IL], BF16)

        preptmp = ctx.enter_context(tc.tile_pool(name="preptmp", bufs=2))
        bandp = ctx.enter_context(tc.tile_pool(name="bandp", bufs=3))
        build = ctx.enter_context(tc.tile_pool(name="build", bufs=3))
        psump = ctx.enter_context(tc.tile_pool(name="psump", bufs=8, space="PSUM"))
        normp = ctx.enter_context(tc.tile_pool(name="normp", bufs=2))

        def prep_blk(blk):
            """wy/wx + in-bounds mask + interleave split for one 128-row block.
            All writes are contiguous; the (i,j)->i+IL*j interleave is a
            strided READ (strided DVE writes measured ~16x slower)."""
            rows = min(128, H - 128 * blk)
            wy = preptmp.tile([128, W], F32, tag="wy")
            wx = preptmp.tile([128, W], F32, tag="wx")
            ta = preptmp.tile([128, W], F32, tag="ta")
            tb = preptmp.tile([128, W], F32, tag="tb")
            tv = preptmp.tile([128, W], F32, tag="tv")
            nc.vector.tensor_scalar(out=wy[:rows], in0=in_f[:rows, 0, blk], scalar1=dt,
                                    scalar2=y0f[:rows, blk:blk + 1], op0=Alu.mult, op1=Alu.add)
            if PREP_ON_GPSIMD:
                # Pool ALU lacks not_equal / stt; use an arithmetic OOB test:
                # e = relu(wy-(H-1)) + relu(-wy) + relu(wx-(W-1)) + relu(-wx)
                # is > 0 iff the point leaves the image (bounds inclusive,
                # exact at the boundary since relu(0) == 0), and whenever it
                # is nonzero it is >= one f32 ulp of the coordinate (~3e-5),
                # so min(e*1e30, BIG) saturates to BIG for every violator.
                g = nc.gpsimd
                g.tensor_scalar(out=wx[:rows], in0=in_f[:rows, 1, blk], scalar1=dt,
                                scalar2=0.0, op0=Alu.mult, op1=Alu.add)
                g.tensor_tensor(out=wx[:rows], in0=wx[:rows], in1=x0f[:rows], op=Alu.add)
                g.tensor_scalar(out=ta[:rows], in0=wy[:rows], scalar1=-float(H - 1),
                                scalar2=0.0, op0=Alu.add, op1=Alu.max)
                g.tensor_scalar(out=tb[:rows], in0=wy[:rows], scalar1=-1.0,
                                scalar2=0.0, op0=Alu.mult, op1=Alu.max)
                g.tensor_tensor(out=ta[:rows], in0=ta[:rows], in1=tb[:rows], op=Alu.add)
                g.tensor_scalar(out=tb[:rows], in0=wx[:rows], scalar1=-float(W - 1),
                                scalar2=0.0, op0=Alu.add, op1=Alu.max)
                g.tensor_scalar(out=tv[:rows], in0=wx[:rows], scalar1=-1.0,
                                scalar2=0.0, op0=Alu.mult, op1=Alu.max)
                g.tensor_tensor(out=tb[:rows], in0=tb[:rows], in1=tv[:rows], op=Alu.add)
                g.tensor_tensor(out=ta[:rows], in0=ta[:rows], in1=tb[:rows], op=Alu.add)
                g.tensor_scalar(out=tv[:rows], in0=ta[:rows], scalar1=1.0e30,
                                scalar2=BIG, op0=Alu.mult, op1=Alu.min)
                g.tensor_tensor(out=ta[:rows], in0=tv[:rows], in1=wy[:rows], op=Alu.add)
            else:
                nc.vector.scalar_tensor_tensor(out=wx[:rows], in0=in_f[:rows, 1, blk], scalar=dt,
                                               in1=x0f[:rows], op0=Alu.mult, op1=Alu.add)
                # violation count: (clamp(wy) != wy) + (clamp(wx) != wx); exact
                # float compare, bounds inclusive as in the reference
                nc.vector.tensor_scalar(out=ta[:rows], in0=wy[:rows], scalar1=0.0,
                                        scalar2=float(H - 1), op0=Alu.max, op1=Alu.min)
                nc.vector.tensor_tensor(out=ta[:rows], in0=ta[:rows], in1=wy[:rows], op=Alu.not_equal)
                nc.vector.tensor_scalar(out=tb[:rows], in0=wx[:rows], scalar1=0.0,
                                        scalar2=float(W - 1), op0=Alu.max, op1=Alu.min)
                nc.vector.tensor_tensor(out=tb[:rows], in0=tb[:rows], in1=wx[:rows], op=Alu.not_equal)
                nc.vector.tensor_tensor(out=tv[:rows], in0=ta[:rows], in1=tb[:rows], op=Alu.add)
                # masked wy (contiguous DVE op)
                nc.vector.scalar_tensor_tensor(out=ta[:rows], in0=tv[:rows], scalar=BIG,
                                               in1=wy[:rows], op0=Alu.mult, op1=Alu.add)

            def rd(t):  # strided read view: dest (i, j) <- src col i + IL*j
                return _v(t[:rows], [[1, IL], [IL, W // IL]])
            nc.gpsimd.tensor_copy(out=PSc[:rows, 0, blk], in_=rd(ta))
            nc.gpsimd.tensor_copy(out=PSc[:rows, 1, blk], in_=rd(wx))
            nc.gpsimd.tensor_copy(out=PSv[:rows, 0, blk],
                                  in_=_v(in_f[:rows, 0, blk], [[1, IL], [IL, W // IL]]))
            nc.gpsimd.tensor_copy(out=PSv[:rows, 1, blk],
                                  in_=_v(in_f[:rows, 1, blk], [[1, IL], [IL, W // IL]]))

        def _norm_cols(b, c0, c1, tag, last):
            """Normalize + store columns [c0,c1) of one storage block."""
            cw = c1 - c0
            rec = normp.tile([128, cw], F32, tag="rec" + tag)
            ofx = normp.tile([128, cw], F32, tag="ofx" + tag)
            ofy = normp.tile([128, cw], F32, tag="ofy" + tag)
            nc.scalar.activation(out=rec[:, :cw], in_=grid[:, b, c0:c1, 0],
                                 func=Act.Identity, bias=epsb[:, 0:1])
            # approx is plenty: ~18 correct bits vs the 2e-2 gate; input is
            # sum(w)+1e-9 >= 1e-9, always a normal f32 (no undefined edges)
            nc.vector.reciprocal_approx_fast(out=rec[:, :cw], in_=rec[:, :cw])
            nc.gpsimd.tensor_tensor(out=ofx[:, :cw], in0=grid[:, b, c0:c1, 2],
                                    in1=rec[:, :cw], op=Alu.mult)
            # on the last block the DVE is idle; run the second mul there in
            # parallel with GpSimd instead of serializing both on GpSimd
            eng = nc.vector if last else nc.gpsimd
            eng.tensor_tensor(out=ofy[:, :cw], in0=grid[:, b, c0:c1, 1],
                              in1=rec[:, :cw], op=Alu.mult)
            p0 = PAD if b == 0 else 0
            pe = min(128, H + PAD - 128 * b)
            r0 = 128 * b + p0 - PAD
            nc.sync.dma_start(out=_v(o_wfx.ap()[r0:r0 + pe - p0], [[1, cw]], extra_off=c0),
                              in_=ofx[p0:pe, :cw])
            nc.scalar.dma_start(out=_v(o_wfy.ap()[r0:r0 + pe - p0], [[1, cw]], extra_off=c0),
                                in_=ofy[p0:pe, :cw])

        def norm_blk(b, last=False):
            """Normalize + store one storage block (undoing +PAD). The last
            block's chain is exposed in the tail, so it is column-halved to
            pipeline the left half's store under the right half's compute."""
            if last:
                _norm_cols(b, 0, W // 2, "L", last)
                _norm_cols(b, W // 2, W, "R", last)
            else:
                _norm_cols(b, 0, W, "", last)

        def emit_spill(st):
            """Deferred spill of one segment's PSUM (software-pipelined one
            segment behind so the matmul->convert->add chain never blocks the
            strict-FIFO DVE/ACT queues at the head)."""
            pseg, a, mY, xlo, XTs = st
            scr = build.tile([128, XT3MAX], BF16, tag="scr")
            c0 = max(0, xlo)
            c1 = min(W, xlo + XTs)
            ncols = c1 - c0
            s1 = min(a + mY, H + PAD)
            y = a
            while y < s1:
                gblk, gp = divmod(y, 128)
                pr = y - a
                # engine partition ranges: start 0 -> <=128, start 64 ->
                # <=64, start 32/96 -> <=32 (can't cross the next quadrant);
                # applies to both the psum side (pr) and the grid side (gp)
                rule = lambda p: 128 - p if p % 64 == 0 else 64 - p % 64
                ln = min(s1 - y, rule(gp), rule(pr))
                nc.scalar.activation(
                    out=_v(scr[gp:gp + ln], [[1, 3 * ncols]],
                           extra_off=(c0 - xlo) * 3),
                    in_=_v(pseg[pr:pr + ln], [[1, 3 * ncols]],
                           extra_off=(c0 - xlo) * 3),
                    func=Act.Copy)
                add_eng.tensor_tensor(
                    out=grid[gp:gp + ln, gblk, c0:c1],
                    in0=_v(scr[gp:gp + ln], [[1, 3 * ncols]],
                           extra_off=(c0 - xlo) * 3),
                    in1=grid[gp:gp + ln, gblk, c0:c1],
                    op=Alu.add)
                y += ln

        # ---- main banded splat, block by block ----
        # Software pipelining: spills trail their segment by 2 (the
        # matmul->ACT-convert chain is fully hidden); the next block's prep is
        # issued during the current block's last band; a block's normalize is
        # deferred into the middle of the NEXT block (grid deps are tracked by
        # tiles, issue order only affects FIFO head-of-line stalls).
        pending = []
        pending_norm = None
        prep_blk(0)
        for blk in range(NBLK):
            blk_bands = [bi for bi, (a, _) in enumerate(bands) if a // 128 == blk]
            for idx, bi in enumerate(blk_bands):
                a, bh = bands[bi]
                p0 = a % 128
                kk = IL * bh
                bandC = bandp.tile([128, 2, W // IL], F32, tag="bandC")   # wyM, wx
                bandV = bandp.tile([128, 2, W // IL], BF16, tag="bandV")  # fy, fx
                for i in range(IL):
                    eng = nc.sync if i % 2 == 0 else nc.scalar
                    eng.dma_start(out=bandC[bh * i:bh * (i + 1)], in_=PSc[p0:p0 + bh, :, blk, i])
                    eng.dma_start(out=bandV[bh * i:bh * (i + 1)], in_=PSv[p0:p0 + bh, :, blk, i])
                if idx == len(blk_bands) - 1 and blk + 1 < NBLK:
                    prep_blk(blk + 1)  # prefetch next block's prep (issued
                    # after this band's loads to avoid a per-tile WAR stall)
                for s in range(nsegs):
                    SPs = min(SP, npairs - SP * s)
                    dyp = dyp_map[bi][s]
                    dxn = dxn_map[bi][s]
                    dxp = dxp_map[bi][s]
                    mY = PAD + bh + dyp + 1
                    XW = dxn + dxp + 2 + IL
                    XTs = IL * SPs + dxn + dxp + 2
                    xlo = IL * SP * s - (dxn + 1)
                    j0 = SP * s

                    tentY = build.tile([128, SP, YWMAX], BF16, tag="tentY")
                    rhs = build.tile([128, SP, 3, XWMAX], BF16, tag="rhs")
                    fvE = build.tile([128, SP, 2, XWMAX], BF16, tag="fvE")

                    # Y tents: tentY = relu(1 - |ioY - wy|), one fused DVE pass
                    nc.vector._custom_dve(
                        TENT,
                        out=tentY[:kk, :SPs, :mY],
                        in0=_v(ioY[:, a:a + mY], [[0, SPs], [1, mY]], parts=kk),
                        in1=_v(bandC[:, 0, j0:j0 + SPs], [[1, SPs], [0, mY]], parts=kk))
                    # X tents into rhs channel 0 (contiguous)
                    nc.vector._custom_dve(
                        TENT,
                        out=rhs[:kk, :SPs, 0, :XW],
                        in0=_v(ioX[:, IL * j0 + dxnmax - dxn:], [[IL, SPs], [1, XW]], parts=kk),
                        in1=_v(bandC[:, 1, j0:j0 + SPs], [[1, SPs], [0, XW]], parts=kk))
                    # expand the (fy, fx) channel pair on the ACT engine in one
                    # pass so the fused DVE mul sees contiguous operands (2x
                    # bf16 mode); ACT casts f32 -> bf16 on the fly
                    nc.scalar.activation(
                        out=fvE[:kk, :SPs, :, :XW],
                        in_=_v(bandV[:], [[1, SPs], [W // IL, 2], [0, XW]],
                               extra_off=j0, parts=kk),
                        func=Act.Copy)
                    # both rhs channels in one fused 2x op: rhs[:, :, 1+c, :] =
                    # rhs[:, :, 0, :] * fvE[:, :, c, :]
                    mul_eng.tensor_tensor(
                        out=rhs[:kk, :SPs, 1:3, :XW],
                        in0=_v(rhs[:], [[3 * XWMAX, SPs], [0, 2], [1, XW]], parts=kk),
                        in1=_v(fvE[:], [[2 * XWMAX, SPs], [XWMAX, 2], [1, XW]], parts=kk),
                        op=Alu.mult)

                    # drain eagerly on the very last band so the end-of-loop
                    # flush holds one entry instead of three stacked waits
                    last_band = (blk == NBLK - 1 and idx == len(blk_bands) - 1)
                    while len(pending) >= (1 if last_band else 3):
                        emit_spill(pending.pop(0))

                    pseg = psump.tile([128, XT3MAX], F32, tag="pseg")
                    # start=True zero matmul: clears the bank's has_written bits and
                    # writes 0 over the full extent, so the sliding accumulation
                    # below is well-defined per element.
                    nc.tensor.matmul(pseg[:mY, :XTs * 3], lhsT=z_l[:, :mY],
                                     rhs=z_r[:, :XTs * 3], start=True, stop=False)
                    for jj in range(SPs):
                        # rhs chunk read ch-inner (x outer, ch inner) to match psum
                        rhs_j = _v(rhs[:kk], [[1, XW], [XWMAX, 3]],
                                   extra_off=jj * 3 * XWMAX)
                        nc.tensor.matmul(
                            pseg[:mY, 3 * IL * jj:3 * IL * jj + XW * 3],
                            lhsT=tentY[:kk, jj, :mY],
                            rhs=rhs_j,
                            start=False, stop=(jj == SPs - 1))

                    # spill deferred two segments (see emit_spill)
                    pending.append((pseg, a, mY, xlo, XTs))
                if idx == 1 and pending_norm is not None:
                    norm_blk(pending_norm)
                    pending_norm = None
            # all spills into storage block `blk` are issued within the first
            # band of block blk+1 (deque depth 2); its normalize is deferred
            # there too so the flush never stalls the DVE queue head
            pending_norm = blk
        for st in pending:
            emit_spill(st)
        norm_blk(NBLK - 1, last=True)

    nc.compile()
    return nc


_PROG_CACHE = {}


def _get_program(disp, dt, dyp_map, dxn_map, dxp_map, SP, XT3, H=H, W=W):
    key = (disp, float(dt), tuple(tuple(r) for r in dyp_map),
           tuple(tuple(r) for r in dxn_map), tuple(tuple(r) for r in dxp_map),
           SP, XT3, H, W, MUL_ON_GPSIMD, ADD_ON_GPSIMD, PREP_ON_GPSIMD)
    if key not in _PROG_CACHE:
        _PROG_CACHE[key] = _build_program(disp, dt, dyp_map, dxn_map, dxp_map, SP, XT3, H=H, W=W)
    return _PROG_CACHE[key]


def _odd(v):
    return v if v % 2 == 1 else v + 1


def _window_params(fy, fx, dt, H=H, W=W):
    """Exact per-region displacement bounds (over all batch elements).
    Returns (disp, dyp_map, dxn_map, dxp_map, SP, XT3): disp = global |dy| max
    (top slack, keeps spills 32-aligned); per-(band,seg) bottom-y max and
    left/right x maxima (x maxima rounded up to odd so every spill/mul
    access stays 4-byte aligned for the DVE 2x mode); XT3 = psum extent."""
    dy = (dt * fy).max(axis=0)                     # [H, W] signed maxima
    dyn_ = (-(dt * fy)).max(axis=0)
    dxp_ = (dt * fx).max(axis=0)
    dxn_ = (-(dt * fx)).max(axis=0)
    disp = max(2, int(math.ceil(float(np.maximum(dy, dyn_).max()))))
    dxn_g = _odd(max(1, int(math.ceil(float(dxn_.max())))))
    dxp_g = _odd(max(1, int(math.ceil(float(dxp_.max())))))
    bands = [(a, min(BH, H - a)) for a in range(0, H, BH)]
    npairs = W // IL

    def build_maps(SPv):
        nsegs = (npairs + SPv - 1) // SPv
        dyp_map, dxn_map, dxp_map = [], [], []
        xt3 = 0
        for (a, bh) in bands:
            ry, rn, rp = [], [], []
            for s in range(nsegs):
                c0 = IL * SPv * s
                c1 = min(W, IL * SPv * (s + 1))
                ry.append(max(1, int(math.ceil(float(dy[a:a + bh, c0:c1].max())))))
                rn.append(min(dxn_g, _odd(max(1, int(math.ceil(float(dxn_[a:a + bh, c0:c1].max())))))))
                rp.append(min(dxp_g, _odd(max(1, int(math.ceil(float(dxp_[a:a + bh, c0:c1].max())))))))
                xt3 = max(xt3, (IL * ((c1 - c0) // IL) + rn[-1] + rp[-1] + 2) * 3)
            dyp_map.append(ry)
            dxn_map.append(rn)
            dxp_map.append(rp)
        return dyp_map, dxn_map, dxp_map, xt3

    # prefer SP=32 (5 uniform segments at W=640); psum is sized by the
    # regional window maxima, not the global bound, so this usually fits
    for SPv in (32,):
        dyp_map, dxn_map, dxp_map, xt3 = build_maps(SPv)
        if xt3 <= 512:
            return disp, dyp_map, dxn_map, dxp_map, SPv, xt3
    # fallback: global-bound sizing (always fits by construction)
    SPv = min(64, (512 // 3 - dxn_g - dxp_g - 2) // IL)
    dyp_map, dxn_map, dxp_map, xt3 = build_maps(SPv)
    return disp, dyp_map, dxn_map, dxp_map, SPv, min(512, xt3)


def kernel(flow_maps_x, flow_maps_y, i=0, tref=4):
    i = int(i)
    tref = int(tref)
    dt = float(tref - i)
    B = flow_maps_x.shape[0]
    assert B <= NCORES, f"batch {B} > {NCORES} cores not supported"
    fx = np.ascontiguousarray(flow_maps_x[:, i]).astype(np.float32)
    fy = np.ascontiguousarray(flow_maps_y[:, i]).astype(np.float32)

    disp, dyp_map, dxn_map, dxp_map, SPv, xt3 = _window_params(fy, fx, dt)
    nc = _get_program(disp, dt, dyp_map, dxn_map, dxp_map, SPv, xt3)
    in_maps = [{"fy": fy[b], "fx": fx[b]} for b in range(B)]
    res = run_bass_kernel_spmd(nc, in_maps, list(range(B)))
    wfx = np.stack([res.results[b]["out_wfx"] for b in range(B)])[:, None]
    wfy = np.stack([res.results[b]["out_wfy"] for b in range(B)])[:, None]
    return wfx.astype(np.float32), wfy.astype(np.float32)


def _ensure_ntff_hook():
    """The agent image lacks antenv.axon_hooks; synthesize it from trn_agent_boot."""
    import types
    try:
        import antenv.axon_hooks  # noqa: F401
        return
    except ImportError:
        pass
    from trn_agent_boot.trn_boot import _ntff_profile_via_ctypes
    hook = _ntff_profile_via_ctypes("/opt/axon/libaxon_pjrt.so")
    m = types.ModuleType("antenv.axon_hooks")
    m.get_axon_ntff_profile_hook = lambda: hook
    m.set_axon_ntff_profile_hook = lambda h: None
    sys.modules["antenv.axon_hooks"] = m


def timed_run(np_inputs):
    """Run once with NTFF tracing; return HW exec time in ns (max over traced cores)."""
    _ensure_ntff_hook()
    i = int(np_inputs["i"]); tref = int(np_inputs["tref"])
    dt = float(tref - i)
    fx = np.ascontiguousarray(np_inputs["flow_maps_x"][:, i]).astype(np.float32)
    fy = np.ascontiguousarray(np_inputs["flow_maps_y"][:, i]).astype(np.float32)
    B = fx.shape[0]
    disp, dyp_map, dxn_map, dxp_map, SPv, xt3 = _window_params(fy, fx, dt)
    nc = _get_program(disp, dt, dyp_map, dxn_map, dxp_map, SPv, xt3)
    in_maps = [{"fy": fy[b], "fx": fx[b]} for b in range(B)]
    res = run_bass_kernel_spmd(nc, in_maps, list(range(B)), trace=True)
    return res.exec_time_ns


if __name__ == "__main__":
    rng = np.random.default_rng(0)
    fmx = rng.standard_normal((8, 4, H, W), dtype=np.float32)
    fmy = rng.standard_normal((8, 4, H, W), dtype=np.float32)
    ox, oy = kernel(fmx, fmy, 0, 4)
    print(ox.shape, oy.shape, ox.dtype)


# revision 24
# speedup vs baseline: 1.0064x; 1.0064x over previous
"""Trainium2 Bass kernel for bilinear forward-warp splatting (scatter_memory).

Computes, per batch element b (data-parallel over 8 NeuronCores):
    wy = y0 + dt*fy;  wx = x0 + dt*fx          (dt = tref - i)
    out[y, x] = sum_p v_p * tent(wy_p - y) * tent(wx_p - x)
for the three channels v in {1, fy, fx}, where tent(u) = max(0, 1-|u|) is
exactly the bilinear splat weight, followed by wf = splat(w*f)/(splat(w)+eps).

Algorithm: displacement dt*flow is bounded (verified on host per region), so
the scatter is a windowed rank-1 accumulation: for each chunk of 128 points
(32 rows x 4 columns of the grid), build a sparse "tent" matrix over the
y-window (lhsT [128, mY]) and the x-window x 3 channels (rhs) with a single
fused custom-DVE op (relu(1-|iota-w|)), then let the TensorEngine accumulate
sum_p tentY[p,:]^T (x) rhs[p,:] into PSUM, sliding along 32-row bands.
PSUM segments are converted to bf16 on the ACT engine and added (2x DVE mode)
into bf16 SBUF grid accumulators.  Windows are per-(band,segment): the y
bottom and both x sides are trimmed to the regional displacement maxima.
W=640 splits into 5 uniform PSUM segments of 32 column-groups (psum sized by
the regional window maxima, not the global bound).  Prep (warp coords,
in-bounds mask, interleave split into band point order) runs per 128-row
block right before its bands with contiguous DVE writes + GpSimd strided
reads; per segment, the (fy, fx) channel pair is broadcast-expanded in ONE
ACT pass and both rhs channel muls run as ONE fused 2x-mode DVE op.
Normalization + output DMA run per 128-row storage block as soon as its last
contributing band has been spilled, overlapping the main loop.

Perf notes (traced on HW): the DVE is the saturated engine (~93%); the tent
ops are f32-in so they run at 1x (fp16 coords would be 2x but coordinate
rounding near integer boundaries flips lone tiny-weight pixels => rel err
0.7, measured); GpSimd elementwise is 6-14 cyc/elem AND its SBUF traffic
slows 2-port DVE ops ~46% (shared port), so offloading DVE work there is a
net loss; strided (stride-16B) SBUF-to-SBUF DMA runs at element rate
(~6.4us per 20KB), so band loads must stage through the GpSimd interleave
copies + contiguous DMA.
"""

import os
import sys
import math

import numpy as np

for _p in ("/opt/trn_rl_repo", "/root/.axon_site/_ro/trn_rl_repo"):
    if os.path.isdir(_p) and _p not in sys.path:
        sys.path.insert(0, _p)

from contextlib import ExitStack

import concourse.bass as bass
import concourse.bacc as bacc
import concourse.tile as tile
from concourse import mybir
from concourse.ap import AP
from concourse.bass_utils import run_bass_kernel_spmd

H, W = 480, 640
NCORES = 8
F32 = mybir.dt.float32
BF16 = mybir.dt.bfloat16  # bf16: f32 exponent range (tent weights down to ~1e-9 must not flush to 0)
Alu = mybir.AluOpType
Act = mybir.ActivationFunctionType

BH = 32          # band height (rows per band)
IL = 4           # column interleave; chunks are BH rows x IL cols = 128 points
EPS = 1e-9
BIG = 4.0e6      # pushed onto wy for masked-out points -> tent == 0 everywhere

# engine assignment knobs (A/B-tunable): which engine runs the fused rhs
# channel muls, the spill grid-accumulate add, and the prep mask chain
MUL_ON_GPSIMD = False
ADD_ON_GPSIMD = False
PREP_ON_GPSIMD = False

_TENT_OP = None


def _tent_op():
    """Register (once) the fused tent op: out = relu(1 - |in0 - in1|)."""
    global _TENT_OP
    if _TENT_OP is not None:
        return _TENT_OP
    from concourse import dve_ops as dvo
    from concourse.dve_spec import Spec, Src0, Src1, One, maxx, relu, lower
    from concourse.dve_uop import DveOpSpec

    name = "TENT_ANT"
    for op in dvo.OPS:
        if op.name == name:
            _TENT_OP = op
            return op
    spec = Spec(
        body=relu(One - maxx(Src0 - Src1, Src1 - Src0)),
        reference=lambda in0, in1, s0, s1, imm2: np.maximum(
            0.0, 1.0 - np.abs(in0 - in1)
        ),
    )
    row = dvo._CUSTOM_DVE_ROW_BASE + len(dvo.OPS)
    shas = {}
    for ver in ("v3", "v4"):
        shas[ver] = DveOpSpec(
            name=name, opcode=row, uops=lower(spec, ver=ver), rd1_en=True
        ).sha(ver)
    op = dvo.DveOp(name, spec, subdim=False, uops_sha=shas)
    dvo.OPS.append(op)
    dvo._SUB_OPCODE_FOR_NAME[name] = row
    dvo.CUSTOM_DVE_SPECS[name] = spec
    _TENT_OP = op
    return op


def _v(ap, dims, extra_off=0, parts=None):
    """Manual AP view: keep ap's partition pair, replace free dims."""
    ppair = [ap.ap[0][0], ap.ap[0][1] if parts is None else parts]
    return AP(tensor=ap.tensor, offset=ap.offset + extra_off, ap=[ppair] + [list(d) for d in dims])


def _build_program(disp, dt, dyp_map, dxn_map, dxp_map, SP, XT3, H=H, W=W):
    """disp: global y half-window (top slack, fixed for 32-aligned spills);
    dyp_map/dxn_map/dxp_map[band][seg]: per-region bottom-y / left-x / right-x
    displacement bounds; SP: column-groups (of IL columns) per PSUM segment;
    XT3: psum free extent (max over regions, <= 512 f32)."""
    TENT = _tent_op()
    PAD = disp + 1
    dypmax = max(max(r) for r in dyp_map)
    dxnmax = max(max(r) for r in dxn_map)
    dxpmax = max(max(r) for r in dxp_map)
    YWMAX = PAD + BH + dypmax + 1           # max y-window of a band
    bands = [(a, min(BH, H - a)) for a in range(0, H, BH)]
    npairs = W // IL
    nsegs = (npairs + SP - 1) // SP
    XWMAX = dxnmax + dxpmax + 2 + IL
    XT3MAX = XT3                            # psum extent (ch-inner) of a segment
    assert XT3MAX <= 512
    NBLK = (H + 127) // 128                 # 128-row blocks for plane storage
    # Grid accumulator rows are stored SHIFTED by +PAD ("storage row" =
    # real row + PAD) so every PSUM spill lands at a 32-aligned partition
    # start (engines require 32-aligned start partitions). Rows [0, PAD) and
    # [H+PAD, ...) of storage only ever accumulate exact zeros.
    assert PAD <= 32, "grid storage shift assumes PAD <= 32"
    NSBLK = (H + PAD + 127) // 128          # storage blocks for the grid
    for m in (dyp_map, dxn_map, dxp_map):
        assert len(m) == len(bands) and all(len(r) == nsegs for r in m)

    nc = bacc.Bacc("TRN2", target_bir_lowering=False, debug=False)
    fy_in = nc.declare_dram_parameter("fy", [H, W], F32, isOutput=False)
    fx_in = nc.declare_dram_parameter("fx", [H, W], F32, isOutput=False)
    o_wfx = nc.declare_dram_parameter("out_wfx", [H, W], F32, isOutput=True)
    o_wfy = nc.declare_dram_parameter("out_wfy", [H, W], F32, isOutput=True)

    mul_eng = None
    add_eng = None

    with ExitStack() as ctx:
        tc = ctx.enter_context(tile.TileContext(nc))
        mul_eng = nc.gpsimd if MUL_ON_GPSIMD else nc.vector
        add_eng = nc.gpsimd if ADD_ON_GPSIMD else nc.vector
        singles = ctx.enter_context(tc.tile_pool(name="singles", bufs=1))

        # ---- constant ramps (f32 iotas: all values exact below 2^24) ----
        NY = H - BH + YWMAX + 8
        NX = W + XWMAX + 8  # covers all slices
        ioY = singles.tile([128, NY], F32)
        ioX = singles.tile([128, NX], F32)
        x0f = singles.tile([128, W], F32)
        y0f = singles.tile([128, NBLK], F32)
        nc.gpsimd.iota(ioY[:], pattern=[[1, NY]], base=-PAD, channel_multiplier=0,
                       allow_small_or_imprecise_dtypes=True)
        nc.gpsimd.iota(ioX[:], pattern=[[1, NX]], base=-(dxnmax + 1), channel_multiplier=0,
                       allow_small_or_imprecise_dtypes=True)
        nc.gpsimd.iota(x0f[:], pattern=[[1, W]], base=0, channel_multiplier=0,
                       allow_small_or_imprecise_dtypes=True)
        nc.gpsimd.iota(y0f[:], pattern=[[128, NBLK]], base=0, channel_multiplier=1,
                       allow_small_or_imprecise_dtypes=True)

        # ---- grid accumulators (ch-inner: 0=w, 1=w*fy, 2=w*fx), +PAD shift ----
        grid = singles.tile([128, NSBLK, W, 3], BF16)
        nc.vector.memset(grid[:, :, :W // 2], 0.0)
        nc.gpsimd.memset(grid[:, :, W // 2:], 0.0)

        # zero operands for the per-segment PSUM-clearing matmul
        z_l = singles.tile([16, YWMAX], BF16)
        z_r = singles.tile([16, 512], BF16)
        epsb = singles.tile([128, 1], F32)
        nbH = singles.tile([128, 1], F32)
        nbW = singles.tile([128, 1], F32)
        nc.gpsimd.memset(z_l[:], 0.0)
        nc.gpsimd.memset(z_r[:], 0.0)
        nc.gpsimd.memset(epsb[:], EPS)
        nc.gpsimd.memset(nbH[:], -float(H - 1))
        nc.gpsimd.memset(nbW[:], -float(W - 1))

        # ---- inputs (all blocks up front; per-block prep happens in-loop) ----
        # in_f plane 0 = fy, plane 1 = fx
        in_f = singles.tile([128, 2, NBLK, W], F32)
        for blk in range(NBLK):
            rows = min(128, H - 128 * blk)
            rsl = slice(128 * blk, 128 * blk + rows)
            # column-split each plane across both DMA paths so the first
            # block's load (which gates the whole prep chain) lands ~2x faster
            nc.sync.dma_start(out=in_f[:rows, 0, blk, :W // 2],
                              in_=_v(fy_in.ap()[rsl], [[1, W // 2]]))
            nc.scalar.dma_start(out=in_f[:rows, 0, blk, W // 2:],
                                in_=_v(fy_in.ap()[rsl], [[1, W // 2]], extra_off=W // 2))
            nc.scalar.dma_start(out=in_f[:rows, 1, blk, :W // 2],
                                in_=_v(fx_in.ap()[rsl], [[1, W // 2]]))
            nc.sync.dma_start(out=in_f[:rows, 1, blk, W // 2:],
                              in_=_v(fx_in.ap()[rsl], [[1, W // 2]], extra_off=W // 2))

        # split-layout planes the band loads DMA from:
        # PSc: plane 0 = wyM (masked warped y), plane 1 = wx   (f32)
        # PSv: plane 0 = fy, plane 1 = fx                      (bf16)
        PSc = singles.tile([128, 2, NBLK, IL, W // IL], F32)
        PSv = singles.tile([128, 2, NBLK, IL, W // IL], BF16)

        preptmp = ctx.enter_context(tc.tile_pool(name="preptmp", bufs=2))
        bandp = ctx.enter_context(tc.tile_pool(name="bandp", bufs=3))
        build = ctx.enter_context(tc.tile_pool(name="build", bufs=3))
        psump = ctx.enter_context(tc.tile_pool(name="psump", bufs=8, space="PSUM"))
        normp = ctx.enter_context(tc.tile_pool(name="normp", bufs=2))

        def prep_blk(blk):
            """wy/wx + in-bounds mask + interleave split for one 128-row block.
            All writes are contiguous; the (i,j)->i+IL*j interleave is a
            strided READ (strided DVE writes measured ~16x slower)."""
            rows = min(128, H - 128 * blk)
            wy = preptmp.tile([128, W], F32, tag="wy")
            wx = preptmp.tile([128, W], F32, tag="wx")
            ta = preptmp.tile([128, W], F32, tag="ta")
            tb = preptmp.tile([128, W], F32, tag="tb")
            tv = preptmp.tile([128, W], F32, tag="tv")
            nc.vector.tensor_scalar(out=wy[:rows], in0=in_f[:rows, 0, blk], scalar1=dt,
                                    scalar2=y0f[:rows, blk:blk + 1], op0=Alu.mult, op1=Alu.add)
            if PREP_ON_GPSIMD:
                # Pool ALU lacks not_equal / stt; use an arithmetic OOB test:
                # e = relu(wy-(H-1)) + relu(-wy) + relu(wx-(W-1)) + relu(-wx)
                # is > 0 iff the point leaves the image (bounds inclusive,
                # exact at the boundary since relu(0) == 0), and whenever it
                # is nonzero it is >= one f32 ulp of the coordinate (~3e-5),
                # so min(e*1e30, BIG) saturates to BIG for every violator.
                g = nc.gpsimd
                g.tensor_scalar(out=wx[:rows], in0=in_f[:rows, 1, blk], scalar1=dt,
                                scalar2=0.0, op0=Alu.mult, op1=Alu.add)
                g.tensor_tensor(out=wx[:rows], in0=wx[:rows], in1=x0f[:rows], op=Alu.add)
                g.tensor_scalar(out=ta[:rows], in0=wy[:rows], scalar1=-float(H - 1),
                                scalar2=0.0, op0=Alu.add, op1=Alu.max)
                g.tensor_scalar(out=tb[:rows], in0=wy[:rows], scalar1=-1.0,
                                scalar2=0.0, op0=Alu.mult, op1=Alu.max)
                g.tensor_tensor(out=ta[:rows], in0=ta[:rows], in1=tb[:rows], op=Alu.add)
                g.tensor_scalar(out=tb[:rows], in0=wx[:rows], scalar1=-float(W - 1),
                                scalar2=0.0, op0=Alu.add, op1=Alu.max)
                g.tensor_scalar(out=tv[:rows], in0=wx[:rows], scalar1=-1.0,
                                scalar2=0.0, op0=Alu.mult, op1=Alu.max)
                g.tensor_tensor(out=tb[:rows], in0=tb[:rows], in1=tv[:rows], op=Alu.add)
                g.tensor_tensor(out=ta[:rows], in0=ta[:rows], in1=tb[:rows], op=Alu.add)
                g.tensor_scalar(out=tv[:rows], in0=ta[:rows], scalar1=1.0e30,
                                scalar2=BIG, op0=Alu.mult, op1=Alu.min)
                g.tensor_tensor(out=ta[:rows], in0=tv[:rows], in1=wy[:rows], op=Alu.add)
            else:
                nc.vector.scalar_tensor_tensor(out=wx[:rows], in0=in_f[:rows, 1, blk], scalar=dt,
                                               in1=x0f[:rows], op0=Alu.mult, op1=Alu.add)
                # arithmetic OOB test with the relu terms on the (slack) ACT
                # engine: e = relu(wy-(H-1)) + relu(-wy) + relu(wx-(W-1)) +
                # relu(-wx) is > 0 iff the point leaves the image (bounds
                # inclusive, exact at the boundary since relu(0) == 0); any
                # nonzero e is >= one f32 ulp of the coordinate (~3e-5), so
                # wy + e*1e30 >= 3e25 pushes every violator's tent to 0
                # (max e ~ 1.3e3 -> e*1e30 stays finite in f32)
                nc.scalar.activation(out=ta[:rows], in_=wy[:rows], func=Act.Relu,
                                     bias=nbH[:rows, 0:1])
                nc.scalar.activation(out=tb[:rows], in_=wy[:rows], func=Act.Relu,
                                     scale=-1.0)
                nc.vector.tensor_tensor(out=ta[:rows], in0=ta[:rows], in1=tb[:rows], op=Alu.add)
                nc.scalar.activation(out=tb[:rows], in_=wx[:rows], func=Act.Relu,
                                     bias=nbW[:rows, 0:1])
                nc.scalar.activation(out=tv[:rows], in_=wx[:rows], func=Act.Relu,
                                     scale=-1.0)
                nc.vector.tensor_tensor(out=tb[:rows], in0=tb[:rows], in1=tv[:rows], op=Alu.add)
                nc.vector.tensor_tensor(out=tv[:rows], in0=ta[:rows], in1=tb[:rows], op=Alu.add)
                # masked wy (contiguous DVE op)
                nc.vector.scalar_tensor_tensor(out=ta[:rows], in0=tv[:rows], scalar=1.0e30,
                                               in1=wy[:rows], op0=Alu.mult, op1=Alu.add)

            def rd(t):  # strided read view: dest (i, j) <- src col i + IL*j
                return _v(t[:rows], [[1, IL], [IL, W // IL]])
            nc.gpsimd.tensor_copy(out=PSc[:rows, 0, blk], in_=rd(ta))
            nc.gpsimd.tensor_copy(out=PSc[:rows, 1, blk], in_=rd(wx))
            nc.gpsimd.tensor_copy(out=PSv[:rows, 0, blk],
                                  in_=_v(in_f[:rows, 0, blk], [[1, IL], [IL, W // IL]]))
            nc.gpsimd.tensor_copy(out=PSv[:rows, 1, blk],
                                  in_=_v(in_f[:rows, 1, blk], [[1, IL], [IL, W // IL]]))

        def _norm_cols(b, c0, c1, tag, last):
            """Normalize + store columns [c0,c1) of one storage block."""
            cw = c1 - c0
            rec = normp.tile([128, cw], F32, tag="rec" + tag)
            ofx = normp.tile([128, cw], F32, tag="ofx" + tag)
            ofy = normp.tile([128, cw], F32, tag="ofy" + tag)
            nc.scalar.activation(out=rec[:, :cw], in_=grid[:, b, c0:c1, 0],
                                 func=Act.Identity, bias=epsb[:, 0:1])
            # approx is plenty: ~18 correct bits vs the 2e-2 gate; input is
            # sum(w)+1e-9 >= 1e-9, always a normal f32 (no undefined edges)
            nc.vector.reciprocal_approx_fast(out=rec[:, :cw], in_=rec[:, :cw])
            nc.gpsimd.tensor_tensor(out=ofx[:, :cw], in0=grid[:, b, c0:c1, 2],
                                    in1=rec[:, :cw], op=Alu.mult)
            # on the last block the DVE is idle; run the second mul there in
            # parallel with GpSimd instead of serializing both on GpSimd
            eng = nc.vector if last else nc.gpsimd
            eng.tensor_tensor(out=ofy[:, :cw], in0=grid[:, b, c0:c1, 1],
                              in1=rec[:, :cw], op=Alu.mult)
            p0 = PAD if b == 0 else 0
            pe = min(128, H + PAD - 128 * b)
            r0 = 128 * b + p0 - PAD
            nc.sync.dma_start(out=_v(o_wfx.ap()[r0:r0 + pe - p0], [[1, cw]], extra_off=c0),
                              in_=ofx[p0:pe, :cw])
            nc.scalar.dma_start(out=_v(o_wfy.ap()[r0:r0 + pe - p0], [[1, cw]], extra_off=c0),
                                in_=ofy[p0:pe, :cw])

        def norm_blk(b, last=False):
            """Normalize + store one storage block (undoing +PAD). The last
            block's chain is exposed in the tail, so it is column-halved to
            pipeline the left half's store under the right half's compute."""
            if last:
                _norm_cols(b, 0, W // 2, "L", last)
                _norm_cols(b, W // 2, W, "R", last)
            else:
                _norm_cols(b, 0, W, "", last)

        def emit_spill(st):
            """Deferred spill of one segment's PSUM (software-pipelined one
            segment behind so the matmul->convert->add chain never blocks the
            strict-FIFO DVE/ACT queues at the head).

            Rows >= v0 (32-aligned) are VIRGIN: no earlier-emitted spill has
            touched them (previous bands reach at most PAD+dyp_prev+1 < v0
            rows past this band's start, and the following band rounds ITS
            add-region up to +64 too, so later spills always add over rows we
            copy).  For those rows, the exclusive-from-left columns [cL, c1)
            get a direct ACT psum->grid copy with no DVE add; only the
            left-overlap columns [c0, cL) (shared with segment s-1, already
            written) still go through convert+add."""
            pseg, a, mY, xlo, XTs, v0, cL = st
            scr = build.tile([128, XT3MAX], BF16, tag="scr")
            c0 = max(0, xlo)
            c1 = min(W, xlo + XTs)
            ncols = c1 - c0
            s1 = min(a + mY, H + PAD)
            y = a
            while y < s1:
                gblk, gp = divmod(y, 128)
                pr = y - a
                # engine partition ranges: start 0 -> <=128, start 64 ->
                # <=64, start 32/96 -> <=32 (can't cross the next quadrant);
                # applies to both the psum side (pr) and the grid side (gp)
                rule = lambda p: 128 - p if p % 64 == 0 else 64 - p % 64
                ln = min(s1 - y, rule(gp), rule(pr))
                if pr < v0:
                    ln = min(ln, v0 - pr)  # split at the virgin boundary
                    nc.scalar.activation(
                        out=_v(scr[gp:gp + ln], [[1, 3 * ncols]],
                               extra_off=(c0 - xlo) * 3),
                        in_=_v(pseg[pr:pr + ln], [[1, 3 * ncols]],
                               extra_off=(c0 - xlo) * 3),
                        func=Act.Copy)
                    add_eng.tensor_tensor(
                        out=grid[gp:gp + ln, gblk, c0:c1],
                        in0=_v(scr[gp:gp + ln], [[1, 3 * ncols]],
                               extra_off=(c0 - xlo) * 3),
                        in1=grid[gp:gp + ln, gblk, c0:c1],
                        op=Alu.add)
                else:
                    if cL > c0:  # left-overlap columns: convert + add
                        nc.scalar.activation(
                            out=_v(scr[gp:gp + ln], [[1, 3 * (cL - c0)]],
                                   extra_off=(c0 - xlo) * 3),
                            in_=_v(pseg[pr:pr + ln], [[1, 3 * (cL - c0)]],
                                   extra_off=(c0 - xlo) * 3),
                            func=Act.Copy)
                        add_eng.tensor_tensor(
                            out=grid[gp:gp + ln, gblk, c0:cL],
                            in0=_v(scr[gp:gp + ln], [[1, 3 * (cL - c0)]],
                                   extra_off=(c0 - xlo) * 3),
                            in1=grid[gp:gp + ln, gblk, c0:cL],
                            op=Alu.add)
                    # exclusive columns: direct psum -> grid copy (casts)
                    nc.scalar.activation(
                        out=grid[gp:gp + ln, gblk, cL:c1],
                        in_=_v(pseg[pr:pr + ln], [[1, 3 * (c1 - cL)]],
                               extra_off=(cL - xlo) * 3),
                        func=Act.Copy)
                y += ln

        # ---- main banded splat, block by block ----
        # Software pipelining: spills trail their segment by 2 (the
        # matmul->ACT-convert chain is fully hidden); the next block's prep is
        # issued during the current block's last band; a block's normalize is
        # deferred into the middle of the NEXT block (grid deps are tracked by
        # tiles, issue order only affects FIFO head-of-line stalls).
        pending = []
        pending_norm = None
        prep_blk(0)
        for blk in range(NBLK):
            blk_bands = [bi for bi, (a, _) in enumerate(bands) if a // 128 == blk]
            for idx, bi in enumerate(blk_bands):
                a, bh = bands[bi]
                p0 = a % 128
                kk = IL * bh
                bandC = bandp.tile([128, 2, W // IL], F32, tag="bandC")   # wyM, wx
                bandV = bandp.tile([128, 2, W // IL], BF16, tag="bandV")  # fy, fx
                for i in range(IL):
                    eng = nc.sync if i % 2 == 0 else nc.scalar
                    eng.dma_start(out=bandC[bh * i:bh * (i + 1)], in_=PSc[p0:p0 + bh, :, blk, i])
                    eng.dma_start(out=bandV[bh * i:bh * (i + 1)], in_=PSv[p0:p0 + bh, :, blk, i])
                if idx == len(blk_bands) - 1 and blk + 1 < NBLK:
                    prep_blk(blk + 1)  # prefetch next block's prep (issued
                    # after this band's loads to avoid a per-tile WAR stall)
                for s in range(nsegs):
                    SPs = min(SP, npairs - SP * s)
                    dyp = dyp_map[bi][s]
                    dxn = dxn_map[bi][s]
                    dxp = dxp_map[bi][s]
                    mY = PAD + bh + dyp + 1
                    XW = dxn + dxp + 2 + IL
                    XTs = IL * SPs + dxn + dxp + 2
                    xlo = IL * SP * s - (dxn + 1)
                    j0 = SP * s

                    tentY = build.tile([128, SP, YWMAX], BF16, tag="tentY")
                    rhs = build.tile([128, SP, 3, XWMAX], BF16, tag="rhs")
                    fvE = build.tile([128, SP, 2, XWMAX], BF16, tag="fvE")

                    # Y tents: tentY = relu(1 - |ioY - wy|), one fused DVE pass
                    nc.vector._custom_dve(
                        TENT,
                        out=tentY[:kk, :SPs, :mY],
                        in0=_v(ioY[:, a:a + mY], [[0, SPs], [1, mY]], parts=kk),
                        in1=_v(bandC[:, 0, j0:j0 + SPs], [[1, SPs], [0, mY]], parts=kk))
                    # X tents into rhs channel 0 (contiguous)
                    nc.vector._custom_dve(
                        TENT,
                        out=rhs[:kk, :SPs, 0, :XW],
                        in0=_v(ioX[:, IL * j0 + dxnmax - dxn:], [[IL, SPs], [1, XW]], parts=kk),
                        in1=_v(bandC[:, 1, j0:j0 + SPs], [[1, SPs], [0, XW]], parts=kk))
                    # expand the (fy, fx) channel pair on the ACT engine in one
                    # pass so the fused DVE mul sees contiguous operands (2x
                    # bf16 mode); ACT casts f32 -> bf16 on the fly
                    nc.scalar.activation(
                        out=fvE[:kk, :SPs, :, :XW],
                        in_=_v(bandV[:], [[1, SPs], [W // IL, 2], [0, XW]],
                               extra_off=j0, parts=kk),
                        func=Act.Copy)
                    # both rhs channels in one fused 2x op: rhs[:, :, 1+c, :] =
                    # rhs[:, :, 0, :] * fvE[:, :, c, :]
                    mul_eng.tensor_tensor(
                        out=rhs[:kk, :SPs, 1:3, :XW],
                        in0=_v(rhs[:], [[3 * XWMAX, SPs], [0, 2], [1, XW]], parts=kk),
                        in1=_v(fvE[:], [[2 * XWMAX, SPs], [XWMAX, 2], [1, XW]], parts=kk),
                        op=Alu.mult)

                    # drain eagerly on the very last band so the end-of-loop
                    # flush holds one entry instead of three stacked waits
                    last_band = (blk == NBLK - 1 and idx == len(blk_bands) - 1)
                    while len(pending) >= (1 if last_band else 3):
                        emit_spill(pending.pop(0))

                    pseg = psump.tile([128, XT3MAX], F32, tag="pseg")
                    # start=True zero matmul: clears the bank's has_written bits and
                    # writes 0 over the full extent, so the sliding accumulation
                    # below is well-defined per element.
                    nc.tensor.matmul(pseg[:mY, :XTs * 3], lhsT=z_l[:, :mY],
                                     rhs=z_r[:, :XTs * 3], start=True, stop=False)
                    for jj in range(SPs):
                        # rhs chunk read ch-inner (x outer, ch inner) to match psum
                        rhs_j = _v(rhs[:kk], [[1, XW], [XWMAX, 3]],
                                   extra_off=jj * 3 * XWMAX)
                        nc.tensor.matmul(
                            pseg[:mY, 3 * IL * jj:3 * IL * jj + XW * 3],
                            lhsT=tentY[:kk, jj, :mY],
                            rhs=rhs_j,
                            start=False, stop=(jj == SPs - 1))

                    # virgin boundary: rows past the previous band's reach,
                    # rounded UP to the engines' 32-partition alignment (the
                    # round-up is also what guarantees the NEXT band still
                    # adds over every row we direct-copy)
                    v0 = 0 if bi == 0 else min(
                        mY, -(-(PAD + dyp_map[bi - 1][s] + 1) // 32) * 32)
                    # exclusive-from-left column start (segment s-1 already
                    # wrote the overlap [c0, cL))
                    cL = max(0, xlo) if s == 0 else max(
                        max(0, xlo), IL * SP * s + dxp_map[bi][s - 1] + 1)
                    # spill deferred two segments (see emit_spill)
                    pending.append((pseg, a, mY, xlo, XTs, v0, cL))
                if idx == 1 and pending_norm is not None:
                    norm_blk(pending_norm)
                    pending_norm = None
            # all spills into storage block `blk` are issued within the first
            # band of block blk+1 (deque depth 2); its normalize is deferred
            # there too so the flush never stalls the DVE queue head
            pending_norm = blk
        for st in pending:
            emit_spill(st)
        norm_blk(NBLK - 1, last=True)

    nc.compile()
    return nc


_PROG_CACHE = {}


def _get_program(disp, dt, dyp_map, dxn_map, dxp_map, SP, XT3, H=H, W=W):
    key = (disp, float(dt), tuple(tuple(r) for r in dyp_map),
           tuple(tuple(r) for r in dxn_map), tuple(tuple(r) for r in dxp_map),
           SP, XT3, H, W, MUL_ON_GPSIMD, ADD_ON_GPSIMD, PREP_ON_GPSIMD)
    if key not in _PROG_CACHE:
        _PROG_CACHE[key] = _build_program(disp, dt, dyp_map, dxn_map, dxp_map, SP, XT3, H=H, W=W)
    return _PROG_CACHE[key]


def _odd(v):
    return v if v % 2 == 1 else v + 1


def _window_params(fy, fx, dt, H=H, W=W):
    """Exact per-region displacement bounds (over all batch elements).
    Returns (disp, dyp_map, dxn_map, dxp_map, SP, XT3): disp = global |dy| max
    (top slack, keeps spills 32-aligned); per-(band,seg) bottom-y max and
    left/right x maxima (x maxima rounded up to odd so every spill/mul
    access stays 4-byte aligned for the DVE 2x mode); XT3 = psum extent."""
    dy = (dt * fy).max(axis=0)                     # [H, W] signed maxima
    dyn_ = (-(dt * fy)).max(axis=0)
    dxp_ = (dt * fx).max(axis=0)
    dxn_ = (-(dt * fx)).max(axis=0)
    disp = max(2, int(math.ceil(float(np.maximum(dy, dyn_).max()))))
    dxn_g = _odd(max(1, int(math.ceil(float(dxn_.max())))))
    dxp_g = _odd(max(1, int(math.ceil(float(dxp_.max())))))
    bands = [(a, min(BH, H - a)) for a in range(0, H, BH)]
    npairs = W // IL

    def build_maps(SPv):
        nsegs = (npairs + SPv - 1) // SPv
        dyp_map, dxn_map, dxp_map = [], [], []
        xt3 = 0
        for (a, bh) in bands:
            ry, rn, rp = [], [], []
            for s in range(nsegs):
                c0 = IL * SPv * s
                c1 = min(W, IL * SPv * (s + 1))
                ry.append(max(1, int(math.ceil(float(dy[a:a + bh, c0:c1].max())))))
                rn.append(min(dxn_g, _odd(max(1, int(math.ceil(float(dxn_[a:a + bh, c0:c1].max())))))))
                rp.append(min(dxp_g, _odd(max(1, int(math.ceil(float(dxp_[a:a + bh, c0:c1].max())))))))
                xt3 = max(xt3, (IL * ((c1 - c0) // IL) + rn[-1] + rp[-1] + 2) * 3)
            dyp_map.append(ry)
            dxn_map.append(rn)
            dxp_map.append(rp)
        return dyp_map, dxn_map, dxp_map, xt3

    # prefer SP=32 (5 uniform segments at W=640); psum is sized by the
    # regional window maxima, not the global bound, so this usually fits
    for SPv in (32,):
        dyp_map, dxn_map, dxp_map, xt3 = build_maps(SPv)
        if xt3 <= 512:
            return disp, dyp_map, dxn_map, dxp_map, SPv, xt3
    # fallback: global-bound sizing (always fits by construction)
    SPv = min(64, (512 // 3 - dxn_g - dxp_g - 2) // IL)
    dyp_map, dxn_map, dxp_map, xt3 = build_maps(SPv)
    return disp, dyp_map, dxn_map, dxp_map, SPv, min(512, xt3)


def kernel(flow_maps_x, flow_maps_y, i=0, tref=4):
    i = int(i)
    tref = int(tref)
    dt = float(tref - i)
    B = flow_maps_x.shape[0]
    assert B <= NCORES, f"batch {B} > {NCORES} cores not supported"
    fx = np.ascontiguousarray(flow_maps_x[:, i]).astype(np.float32)
    fy = np.ascontiguousarray(flow_maps_y[:, i]).astype(np.float32)

    disp, dyp_map, dxn_map, dxp_map, SPv, xt3 = _window_params(fy, fx, dt)
    nc = _get_program(disp, dt, dyp_map, dxn_map, dxp_map, SPv, xt3)
    in_maps = [{"fy": fy[b], "fx": fx[b]} for b in range(B)]
    res = run_bass_kernel_spmd(nc, in_maps, list(range(B)))
    wfx = np.stack([res.results[b]["out_wfx"] for b in range(B)])[:, None]
    wfy = np.stack([res.results[b]["out_wfy"] for b in range(B)])[:, None]
    return wfx.astype(np.float32), wfy.astype(np.float32)


def _ensure_ntff_hook():
    """The agent image lacks antenv.axon_hooks; synthesize it from trn_agent_boot."""
    import types
    try:
        import antenv.axon_hooks  # noqa: F401
        return
    except ImportError:
        pass
    from trn_agent_boot.trn_boot import _ntff_profile_via_ctypes
    hook = _ntff_profile_via_ctypes("/opt/axon/libaxon_pjrt.so")
    m = types.ModuleType("antenv.axon_hooks")
    m.get_axon_ntff_profile_hook = lambda: hook
    m.set_axon_ntff_profile_hook = lambda h: None
    sys.modules["antenv.axon_hooks"] = m


def timed_run(np_inputs):
    """Run once with NTFF tracing; return HW exec time in ns (max over traced cores)."""
    _ensure_ntff_hook()
    i = int(np_inputs["i"]); tref = int(np_inputs["tref"])
    dt = float(tref - i)
    fx = np.ascontiguousarray(np_inputs["flow_maps_x"][:, i]).astype(np.float32)
    fy = np.ascontiguousarray(np_inputs["flow_maps_y"][:, i]).astype(np.float32)
    B = fx.shape[0]
    disp, dyp_map, dxn_map, dxp_map, SPv, xt3 = _window_params(fy, fx, dt)
    nc = _get_program(disp, dt, dyp_map, dxn_map, dxp_map, SPv, xt3)
    in_maps = [{"fy": fy[b], "fx": fx[b]} for b in range(B)]
    res = run_bass_kernel_spmd(nc, in_maps, list(range(B)), trace=True)
    return res.exec_time_ns


if __name__ == "__main__":
    rng = np.random.default_rng(0)
    fmx = rng.standard_normal((8, 4, H, W), dtype=np.float32)
    fmy = rng.standard_normal((8, 4, H, W), dtype=np.float32)
    ox, oy = kernel(fmx, fmy, 0, 4)
    print(ox.shape, oy.shape, ox.dtype)


# revision 25
# speedup vs baseline: 1.0183x; 1.0118x over previous
"""Trainium2 Bass kernel for bilinear forward-warp splatting (scatter_memory).

Computes, per batch element b (data-parallel over 8 NeuronCores):
    wy = y0 + dt*fy;  wx = x0 + dt*fx          (dt = tref - i)
    out[y, x] = sum_p v_p * tent(wy_p - y) * tent(wx_p - x)
for the three channels v in {1, fy, fx}, where tent(u) = max(0, 1-|u|) is
exactly the bilinear splat weight, followed by wf = splat(w*f)/(splat(w)+eps).

Algorithm: displacement dt*flow is bounded (verified on host per region), so
the scatter is a windowed rank-1 accumulation: for each chunk of 128 points
(32 rows x 4 columns of the grid), build a sparse "tent" matrix over the
y-window (lhsT [128, mY]) and the x-window x 3 channels (rhs) with a single
fused custom-DVE op (relu(1-|iota-w|)), then let the TensorEngine accumulate
sum_p tentY[p,:]^T (x) rhs[p,:] into PSUM, sliding along 32-row bands.
PSUM segments are converted to bf16 on the ACT engine and added (2x DVE mode)
into bf16 SBUF grid accumulators.  Windows are per-(band,segment): the y
bottom and both x sides are trimmed to the regional displacement maxima.
W=640 splits into 5 uniform PSUM segments of 32 column-groups (psum sized by
the regional window maxima, not the global bound).  Prep (warp coords,
in-bounds mask, interleave split into band point order) runs per 128-row
block right before its bands with contiguous DVE writes + GpSimd strided
reads; per segment, the (fy, fx) channel pair is broadcast-expanded in ONE
ACT pass and both rhs channel muls run as ONE fused 2x-mode DVE op.
Normalization + output DMA run per 128-row storage block as soon as its last
contributing band has been spilled, overlapping the main loop.

Perf notes (traced on HW): the DVE is the saturated engine (~93%); the tent
ops are f32-in so they run at 1x (fp16 coords would be 2x but coordinate
rounding near integer boundaries flips lone tiny-weight pixels => rel err
0.7, measured); GpSimd elementwise is 6-14 cyc/elem AND its SBUF traffic
slows 2-port DVE ops ~46% (shared port), so offloading DVE work there is a
net loss; strided (stride-16B) SBUF-to-SBUF DMA runs at element rate
(~6.4us per 20KB), so band loads must stage through the GpSimd interleave
copies + contiguous DMA.
"""

import os
import sys
import math

import numpy as np

for _p in ("/opt/trn_rl_repo", "/root/.axon_site/_ro/trn_rl_repo"):
    if os.path.isdir(_p) and _p not in sys.path:
        sys.path.insert(0, _p)

from contextlib import ExitStack

import concourse.bass as bass
import concourse.bacc as bacc
import concourse.tile as tile
from concourse import mybir
from concourse.ap import AP
from concourse.bass_utils import run_bass_kernel_spmd

H, W = 480, 640
NCORES = 8
F32 = mybir.dt.float32
BF16 = mybir.dt.bfloat16  # bf16: f32 exponent range (tent weights down to ~1e-9 must not flush to 0)
Alu = mybir.AluOpType
Act = mybir.ActivationFunctionType

BH = 32          # band height (rows per band)
IL = 4           # column interleave; chunks are BH rows x IL cols = 128 points
EPS = 1e-9
BIG = 4.0e6      # pushed onto wy for masked-out points -> tent == 0 everywhere

# engine assignment knobs (A/B-tunable): which engine runs the fused rhs
# channel muls, the spill grid-accumulate add, and the prep mask chain
MUL_ON_GPSIMD = False
ADD_ON_GPSIMD = False
PREP_ON_GPSIMD = False

_TENT_OP = None


def _tent_op():
    """Register (once) the fused tent op: out = relu(1 - |in0 - in1|)."""
    global _TENT_OP
    if _TENT_OP is not None:
        return _TENT_OP
    from concourse import dve_ops as dvo
    from concourse.dve_spec import Spec, Src0, Src1, One, maxx, relu, lower
    from concourse.dve_uop import DveOpSpec

    name = "TENT_ANT"
    for op in dvo.OPS:
        if op.name == name:
            _TENT_OP = op
            return op
    spec = Spec(
        body=relu(One - maxx(Src0 - Src1, Src1 - Src0)),
        reference=lambda in0, in1, s0, s1, imm2: np.maximum(
            0.0, 1.0 - np.abs(in0 - in1)
        ),
    )
    row = dvo._CUSTOM_DVE_ROW_BASE + len(dvo.OPS)
    shas = {}
    for ver in ("v3", "v4"):
        shas[ver] = DveOpSpec(
            name=name, opcode=row, uops=lower(spec, ver=ver), rd1_en=True
        ).sha(ver)
    op = dvo.DveOp(name, spec, subdim=False, uops_sha=shas)
    dvo.OPS.append(op)
    dvo._SUB_OPCODE_FOR_NAME[name] = row
    dvo.CUSTOM_DVE_SPECS[name] = spec
    _TENT_OP = op
    return op


def _v(ap, dims, extra_off=0, parts=None):
    """Manual AP view: keep ap's partition pair, replace free dims."""
    ppair = [ap.ap[0][0], ap.ap[0][1] if parts is None else parts]
    return AP(tensor=ap.tensor, offset=ap.offset + extra_off, ap=[ppair] + [list(d) for d in dims])


def _build_program(disp, dt, dyp_map, dxn_map, dxp_map, SP, XT3, H=H, W=W):
    """disp: global y half-window (top slack, fixed for 32-aligned spills);
    dyp_map/dxn_map/dxp_map[band][seg]: per-region bottom-y / left-x / right-x
    displacement bounds; SP: column-groups (of IL columns) per PSUM segment;
    XT3: psum free extent (max over regions, <= 512 f32)."""
    TENT = _tent_op()
    PAD = disp + 1
    dypmax = max(max(r) for r in dyp_map)
    dxnmax = max(max(r) for r in dxn_map)
    dxpmax = max(max(r) for r in dxp_map)
    YWMAX = PAD + BH + dypmax + 1           # max y-window of a band
    bands = [(a, min(BH, H - a)) for a in range(0, H, BH)]
    npairs = W // IL
    nsegs = (npairs + SP - 1) // SP
    XWMAX = dxnmax + dxpmax + 2 + IL
    XT3MAX = XT3                            # psum extent (ch-inner) of a segment
    assert XT3MAX <= 512
    NBLK = (H + 127) // 128                 # 128-row blocks for plane storage
    # Grid accumulator rows are stored SHIFTED by +PAD ("storage row" =
    # real row + PAD) so every PSUM spill lands at a 32-aligned partition
    # start (engines require 32-aligned start partitions). Rows [0, PAD) and
    # [H+PAD, ...) of storage only ever accumulate exact zeros.
    assert PAD <= 32, "grid storage shift assumes PAD <= 32"
    NSBLK = (H + PAD + 127) // 128          # storage blocks for the grid
    for m in (dyp_map, dxn_map, dxp_map):
        assert len(m) == len(bands) and all(len(r) == nsegs for r in m)

    nc = bacc.Bacc("TRN2", target_bir_lowering=False, debug=False)
    fy_in = nc.declare_dram_parameter("fy", [H, W], F32, isOutput=False)
    fx_in = nc.declare_dram_parameter("fx", [H, W], F32, isOutput=False)
    o_wfx = nc.declare_dram_parameter("out_wfx", [H, W], F32, isOutput=True)
    o_wfy = nc.declare_dram_parameter("out_wfy", [H, W], F32, isOutput=True)

    mul_eng = None
    add_eng = None

    with ExitStack() as ctx:
        tc = ctx.enter_context(tile.TileContext(nc))
        mul_eng = nc.gpsimd if MUL_ON_GPSIMD else nc.vector
        add_eng = nc.gpsimd if ADD_ON_GPSIMD else nc.vector
        singles = ctx.enter_context(tc.tile_pool(name="singles", bufs=1))

        # ---- constant ramps (f32 iotas: all values exact below 2^24) ----
        NY = H - BH + YWMAX + 8
        NX = W + XWMAX + 8  # covers all slices
        ioY = singles.tile([128, NY], F32)
        ioX = singles.tile([128, NX], F32)
        x0f = singles.tile([128, W], F32)
        y0f = singles.tile([128, NBLK], F32)
        nc.gpsimd.iota(ioY[:], pattern=[[1, NY]], base=-PAD, channel_multiplier=0,
                       allow_small_or_imprecise_dtypes=True)
        nc.gpsimd.iota(ioX[:], pattern=[[1, NX]], base=-(dxnmax + 1), channel_multiplier=0,
                       allow_small_or_imprecise_dtypes=True)
        nc.gpsimd.iota(x0f[:], pattern=[[1, W]], base=0, channel_multiplier=0,
                       allow_small_or_imprecise_dtypes=True)
        nc.gpsimd.iota(y0f[:], pattern=[[128, NBLK]], base=0, channel_multiplier=1,
                       allow_small_or_imprecise_dtypes=True)

        # ---- grid accumulators (ch-inner: 0=w, 1=w*fy, 2=w*fx), +PAD shift ----
        grid = singles.tile([128, NSBLK, W, 3], BF16)
        nc.vector.memset(grid[:, :, :W // 2], 0.0)
        nc.gpsimd.memset(grid[:, :, W // 2:], 0.0)

        # zero operands for the per-segment PSUM-clearing matmul
        z_l = singles.tile([16, YWMAX], BF16)
        z_r = singles.tile([16, 512], BF16)
        epsb = singles.tile([128, 1], F32)
        nbH = singles.tile([128, 1], F32)
        nbW = singles.tile([128, 1], F32)
        nc.gpsimd.memset(z_l[:], 0.0)
        nc.gpsimd.memset(z_r[:], 0.0)
        nc.gpsimd.memset(epsb[:], EPS)
        nc.gpsimd.memset(nbH[:], -float(H - 1))
        nc.gpsimd.memset(nbW[:], -float(W - 1))

        # ---- inputs (all blocks up front; per-block prep happens in-loop) ----
        # in_f plane 0 = fy, plane 1 = fx
        in_f = singles.tile([128, 2, NBLK, W], F32)
        for blk in range(NBLK):
            rows = min(128, H - 128 * blk)
            rsl = slice(128 * blk, 128 * blk + rows)
            # column-split each plane across both DMA paths so the first
            # block's load (which gates the whole prep chain) lands ~2x faster
            nc.sync.dma_start(out=in_f[:rows, 0, blk, :W // 2],
                              in_=_v(fy_in.ap()[rsl], [[1, W // 2]]))
            nc.scalar.dma_start(out=in_f[:rows, 0, blk, W // 2:],
                                in_=_v(fy_in.ap()[rsl], [[1, W // 2]], extra_off=W // 2))
            nc.scalar.dma_start(out=in_f[:rows, 1, blk, :W // 2],
                                in_=_v(fx_in.ap()[rsl], [[1, W // 2]]))
            nc.sync.dma_start(out=in_f[:rows, 1, blk, W // 2:],
                              in_=_v(fx_in.ap()[rsl], [[1, W // 2]], extra_off=W // 2))

        # split-layout planes the band loads DMA from:
        # PSc: plane 0 = wyM (masked warped y), plane 1 = wx   (f32)
        # PSv: plane 0 = fy, plane 1 = fx                      (bf16)
        PSc = singles.tile([128, 2, NBLK, IL, W // IL], F32)
        PSv = singles.tile([128, 2, NBLK, IL, W // IL], BF16)

        preptmp = ctx.enter_context(tc.tile_pool(name="preptmp", bufs=2))
        bandp = ctx.enter_context(tc.tile_pool(name="bandp", bufs=3))
        build = ctx.enter_context(tc.tile_pool(name="build", bufs=3))
        psump = ctx.enter_context(tc.tile_pool(name="psump", bufs=8, space="PSUM"))
        normp = ctx.enter_context(tc.tile_pool(name="normp", bufs=2))

        def prep_blk(blk):
            """wy/wx + in-bounds mask + interleave split for one 128-row block.
            All writes are contiguous; the (i,j)->i+IL*j interleave is a
            strided READ (strided DVE writes measured ~16x slower)."""
            rows = min(128, H - 128 * blk)
            wy = preptmp.tile([128, W], F32, tag="wy")
            wx = preptmp.tile([128, W], F32, tag="wx")
            ta = preptmp.tile([128, W], F32, tag="ta")
            tb = preptmp.tile([128, W], F32, tag="tb")
            tv = preptmp.tile([128, W], F32, tag="tv")
            nc.vector.tensor_scalar(out=wy[:rows], in0=in_f[:rows, 0, blk], scalar1=dt,
                                    scalar2=y0f[:rows, blk:blk + 1], op0=Alu.mult, op1=Alu.add)
            if PREP_ON_GPSIMD:
                # Pool ALU lacks not_equal / stt; use an arithmetic OOB test:
                # e = relu(wy-(H-1)) + relu(-wy) + relu(wx-(W-1)) + relu(-wx)
                # is > 0 iff the point leaves the image (bounds inclusive,
                # exact at the boundary since relu(0) == 0), and whenever it
                # is nonzero it is >= one f32 ulp of the coordinate (~3e-5),
                # so min(e*1e30, BIG) saturates to BIG for every violator.
                g = nc.gpsimd
                g.tensor_scalar(out=wx[:rows], in0=in_f[:rows, 1, blk], scalar1=dt,
                                scalar2=0.0, op0=Alu.mult, op1=Alu.add)
                g.tensor_tensor(out=wx[:rows], in0=wx[:rows], in1=x0f[:rows], op=Alu.add)
                g.tensor_scalar(out=ta[:rows], in0=wy[:rows], scalar1=-float(H - 1),
                                scalar2=0.0, op0=Alu.add, op1=Alu.max)
                g.tensor_scalar(out=tb[:rows], in0=wy[:rows], scalar1=-1.0,
                                scalar2=0.0, op0=Alu.mult, op1=Alu.max)
                g.tensor_tensor(out=ta[:rows], in0=ta[:rows], in1=tb[:rows], op=Alu.add)
                g.tensor_scalar(out=tb[:rows], in0=wx[:rows], scalar1=-float(W - 1),
                                scalar2=0.0, op0=Alu.add, op1=Alu.max)
                g.tensor_scalar(out=tv[:rows], in0=wx[:rows], scalar1=-1.0,
                                scalar2=0.0, op0=Alu.mult, op1=Alu.max)
                g.tensor_tensor(out=tb[:rows], in0=tb[:rows], in1=tv[:rows], op=Alu.add)
                g.tensor_tensor(out=ta[:rows], in0=ta[:rows], in1=tb[:rows], op=Alu.add)
                g.tensor_scalar(out=tv[:rows], in0=ta[:rows], scalar1=1.0e30,
                                scalar2=BIG, op0=Alu.mult, op1=Alu.min)
                g.tensor_tensor(out=ta[:rows], in0=tv[:rows], in1=wy[:rows], op=Alu.add)
            else:
                nc.vector.scalar_tensor_tensor(out=wx[:rows], in0=in_f[:rows, 1, blk], scalar=dt,
                                               in1=x0f[:rows], op0=Alu.mult, op1=Alu.add)
                # violation count: (clamp(wy) != wy) + (clamp(wx) != wx); exact
                # float compare, bounds inclusive as in the reference.  Kept
                # DVE-local: routing these through the busy strict-FIFO ACT
                # queue delays the prep chain ~12us at every block boundary
                # (measured).
                nc.vector.tensor_scalar(out=ta[:rows], in0=wy[:rows], scalar1=0.0,
                                        scalar2=float(H - 1), op0=Alu.max, op1=Alu.min)
                nc.vector.tensor_tensor(out=ta[:rows], in0=ta[:rows], in1=wy[:rows], op=Alu.not_equal)
                nc.vector.tensor_scalar(out=tb[:rows], in0=wx[:rows], scalar1=0.0,
                                        scalar2=float(W - 1), op0=Alu.max, op1=Alu.min)
                nc.vector.tensor_tensor(out=tb[:rows], in0=tb[:rows], in1=wx[:rows], op=Alu.not_equal)
                nc.vector.tensor_tensor(out=tv[:rows], in0=ta[:rows], in1=tb[:rows], op=Alu.add)
                # masked wy (contiguous DVE op)
                nc.vector.scalar_tensor_tensor(out=ta[:rows], in0=tv[:rows], scalar=BIG,
                                               in1=wy[:rows], op0=Alu.mult, op1=Alu.add)

            def rd(t):  # strided read view: dest (i, j) <- src col i + IL*j
                return _v(t[:rows], [[1, IL], [IL, W // IL]])
            nc.gpsimd.tensor_copy(out=PSc[:rows, 0, blk], in_=rd(ta))
            nc.gpsimd.tensor_copy(out=PSc[:rows, 1, blk], in_=rd(wx))
            nc.gpsimd.tensor_copy(out=PSv[:rows, 0, blk],
                                  in_=_v(in_f[:rows, 0, blk], [[1, IL], [IL, W // IL]]))
            nc.gpsimd.tensor_copy(out=PSv[:rows, 1, blk],
                                  in_=_v(in_f[:rows, 1, blk], [[1, IL], [IL, W // IL]]))

        def _norm_cols(b, c0, c1, tag, last):
            """Normalize + store columns [c0,c1) of one storage block."""
            cw = c1 - c0
            rec = normp.tile([128, cw], F32, tag="rec" + tag)
            ofx = normp.tile([128, cw], F32, tag="ofx" + tag)
            ofy = normp.tile([128, cw], F32, tag="ofy" + tag)
            nc.scalar.activation(out=rec[:, :cw], in_=grid[:, b, c0:c1, 0],
                                 func=Act.Identity, bias=epsb[:, 0:1])
            # approx is plenty: ~18 correct bits vs the 2e-2 gate; input is
            # sum(w)+1e-9 >= 1e-9, always a normal f32 (no undefined edges)
            nc.vector.reciprocal_approx_fast(out=rec[:, :cw], in_=rec[:, :cw])
            nc.gpsimd.tensor_tensor(out=ofx[:, :cw], in0=grid[:, b, c0:c1, 2],
                                    in1=rec[:, :cw], op=Alu.mult)
            # on the last block the DVE is idle; run the second mul there in
            # parallel with GpSimd instead of serializing both on GpSimd
            eng = nc.vector if last else nc.gpsimd
            eng.tensor_tensor(out=ofy[:, :cw], in0=grid[:, b, c0:c1, 1],
                              in1=rec[:, :cw], op=Alu.mult)
            p0 = PAD if b == 0 else 0
            pe = min(128, H + PAD - 128 * b)
            r0 = 128 * b + p0 - PAD
            nc.sync.dma_start(out=_v(o_wfx.ap()[r0:r0 + pe - p0], [[1, cw]], extra_off=c0),
                              in_=ofx[p0:pe, :cw])
            nc.scalar.dma_start(out=_v(o_wfy.ap()[r0:r0 + pe - p0], [[1, cw]], extra_off=c0),
                                in_=ofy[p0:pe, :cw])

        def norm_blk(b, last=False):
            """Normalize + store one storage block (undoing +PAD). The last
            block's chain is exposed in the tail, so it is column-halved to
            pipeline the left half's store under the right half's compute."""
            if last:
                _norm_cols(b, 0, W // 2, "L", last)
                _norm_cols(b, W // 2, W, "R", last)
            else:
                _norm_cols(b, 0, W, "", last)

        def emit_spill(st):
            """Deferred spill of one segment's PSUM (software-pipelined one
            segment behind so the matmul->convert->add chain never blocks the
            strict-FIFO DVE/ACT queues at the head).

            Rows >= v0 (32-aligned) are VIRGIN: no earlier-emitted spill has
            touched them (previous bands reach at most PAD+dyp_prev+1 < v0
            rows past this band's start, and the following band rounds ITS
            add-region up to +64 too, so later spills always add over rows we
            copy).  For those rows, the exclusive-from-left columns [cL, c1)
            get a direct ACT psum->grid copy with no DVE add; only the
            left-overlap columns [c0, cL) (shared with segment s-1, already
            written) still go through convert+add."""
            pseg, a, mY, xlo, XTs, v0, cL = st
            scr = build.tile([128, XT3MAX], BF16, tag="scr")
            c0 = max(0, xlo)
            c1 = min(W, xlo + XTs)
            ncols = c1 - c0
            s1 = min(a + mY, H + PAD)
            y = a
            while y < s1:
                gblk, gp = divmod(y, 128)
                pr = y - a
                # engine partition ranges: start 0 -> <=128, start 64 ->
                # <=64, start 32/96 -> <=32 (can't cross the next quadrant);
                # applies to both the psum side (pr) and the grid side (gp)
                rule = lambda p: 128 - p if p % 64 == 0 else 64 - p % 64
                ln = min(s1 - y, rule(gp), rule(pr))
                if pr < v0:
                    ln = min(ln, v0 - pr)  # split at the virgin boundary
                    nc.scalar.activation(
                        out=_v(scr[gp:gp + ln], [[1, 3 * ncols]],
                               extra_off=(c0 - xlo) * 3),
                        in_=_v(pseg[pr:pr + ln], [[1, 3 * ncols]],
                               extra_off=(c0 - xlo) * 3),
                        func=Act.Copy)
                    add_eng.tensor_tensor(
                        out=grid[gp:gp + ln, gblk, c0:c1],
                        in0=_v(scr[gp:gp + ln], [[1, 3 * ncols]],
                               extra_off=(c0 - xlo) * 3),
                        in1=grid[gp:gp + ln, gblk, c0:c1],
                        op=Alu.add)
                else:
                    if cL > c0:  # left-overlap columns: convert + add
                        nc.scalar.activation(
                            out=_v(scr[gp:gp + ln], [[1, 3 * (cL - c0)]],
                                   extra_off=(c0 - xlo) * 3),
                            in_=_v(pseg[pr:pr + ln], [[1, 3 * (cL - c0)]],
                                   extra_off=(c0 - xlo) * 3),
                            func=Act.Copy)
                        add_eng.tensor_tensor(
                            out=grid[gp:gp + ln, gblk, c0:cL],
                            in0=_v(scr[gp:gp + ln], [[1, 3 * (cL - c0)]],
                                   extra_off=(c0 - xlo) * 3),
                            in1=grid[gp:gp + ln, gblk, c0:cL],
                            op=Alu.add)
                    # exclusive columns: direct psum -> grid copy (casts)
                    nc.scalar.activation(
                        out=grid[gp:gp + ln, gblk, cL:c1],
                        in_=_v(pseg[pr:pr + ln], [[1, 3 * (c1 - cL)]],
                               extra_off=(cL - xlo) * 3),
                        func=Act.Copy)
                y += ln

        # ---- main banded splat, block by block ----
        # Software pipelining: spills trail their segment by 2 (the
        # matmul->ACT-convert chain is fully hidden); the next block's prep is
        # issued during the current block's last band; a block's normalize is
        # deferred into the middle of the NEXT block (grid deps are tracked by
        # tiles, issue order only affects FIFO head-of-line stalls).
        pending = []
        pending_norm = None
        prep_blk(0)
        for blk in range(NBLK):
            blk_bands = [bi for bi, (a, _) in enumerate(bands) if a // 128 == blk]
            for idx, bi in enumerate(blk_bands):
                a, bh = bands[bi]
                p0 = a % 128
                kk = IL * bh
                bandC = bandp.tile([128, 2, W // IL], F32, tag="bandC")   # wyM, wx
                bandV = bandp.tile([128, 2, W // IL], BF16, tag="bandV")  # fy, fx
                for i in range(IL):
                    eng = nc.sync if i % 2 == 0 else nc.scalar
                    eng.dma_start(out=bandC[bh * i:bh * (i + 1)], in_=PSc[p0:p0 + bh, :, blk, i])
                    eng.dma_start(out=bandV[bh * i:bh * (i + 1)], in_=PSv[p0:p0 + bh, :, blk, i])
                if idx == len(blk_bands) - 1 and blk + 1 < NBLK:
                    prep_blk(blk + 1)  # prefetch next block's prep (issued
                    # after this band's loads to avoid a per-tile WAR stall)
                for s in range(nsegs):
                    SPs = min(SP, npairs - SP * s)
                    dyp = dyp_map[bi][s]
                    dxn = dxn_map[bi][s]
                    dxp = dxp_map[bi][s]
                    mY = PAD + bh + dyp + 1
                    XW = dxn + dxp + 2 + IL
                    XTs = IL * SPs + dxn + dxp + 2
                    xlo = IL * SP * s - (dxn + 1)
                    j0 = SP * s

                    tentY = build.tile([128, SP, YWMAX], BF16, tag="tentY")
                    rhs = build.tile([128, SP, 3, XWMAX], BF16, tag="rhs")
                    fvE = build.tile([128, SP, 2, XWMAX], BF16, tag="fvE")

                    # Y tents: tentY = relu(1 - |ioY - wy|), one fused DVE pass
                    nc.vector._custom_dve(
                        TENT,
                        out=tentY[:kk, :SPs, :mY],
                        in0=_v(ioY[:, a:a + mY], [[0, SPs], [1, mY]], parts=kk),
                        in1=_v(bandC[:, 0, j0:j0 + SPs], [[1, SPs], [0, mY]], parts=kk))
                    # X tents into rhs channel 0 (contiguous)
                    nc.vector._custom_dve(
                        TENT,
                        out=rhs[:kk, :SPs, 0, :XW],
                        in0=_v(ioX[:, IL * j0 + dxnmax - dxn:], [[IL, SPs], [1, XW]], parts=kk),
                        in1=_v(bandC[:, 1, j0:j0 + SPs], [[1, SPs], [0, XW]], parts=kk))
                    # expand the (fy, fx) channel pair on the ACT engine in one
                    # pass so the fused DVE mul sees contiguous operands (2x
                    # bf16 mode); ACT casts f32 -> bf16 on the fly
                    nc.scalar.activation(
                        out=fvE[:kk, :SPs, :, :XW],
                        in_=_v(bandV[:], [[1, SPs], [W // IL, 2], [0, XW]],
                               extra_off=j0, parts=kk),
                        func=Act.Copy)
                    # both rhs channels in one fused 2x op: rhs[:, :, 1+c, :] =
                    # rhs[:, :, 0, :] * fvE[:, :, c, :]
                    mul_eng.tensor_tensor(
                        out=rhs[:kk, :SPs, 1:3, :XW],
                        in0=_v(rhs[:], [[3 * XWMAX, SPs], [0, 2], [1, XW]], parts=kk),
                        in1=_v(fvE[:], [[2 * XWMAX, SPs], [XWMAX, 2], [1, XW]], parts=kk),
                        op=Alu.mult)

                    # drain eagerly on the very last band so the end-of-loop
                    # flush holds one entry instead of three stacked waits
                    last_band = (blk == NBLK - 1 and idx == len(blk_bands) - 1)
                    while len(pending) >= (1 if last_band else 3):
                        emit_spill(pending.pop(0))

                    pseg = psump.tile([128, XT3MAX], F32, tag="pseg")
                    # start=True zero matmul: clears the bank's has_written bits and
                    # writes 0 over the full extent, so the sliding accumulation
                    # below is well-defined per element.
                    nc.tensor.matmul(pseg[:mY, :XTs * 3], lhsT=z_l[:, :mY],
                                     rhs=z_r[:, :XTs * 3], start=True, stop=False)
                    for jj in range(SPs):
                        # rhs chunk read ch-inner (x outer, ch inner) to match psum
                        rhs_j = _v(rhs[:kk], [[1, XW], [XWMAX, 3]],
                                   extra_off=jj * 3 * XWMAX)
                        nc.tensor.matmul(
                            pseg[:mY, 3 * IL * jj:3 * IL * jj + XW * 3],
                            lhsT=tentY[:kk, jj, :mY],
                            rhs=rhs_j,
                            start=False, stop=(jj == SPs - 1))

                    # virgin boundary: rows past the previous band's reach,
                    # rounded UP to the engines' 32-partition alignment (the
                    # round-up is also what guarantees the NEXT band still
                    # adds over every row we direct-copy)
                    v0 = 0 if bi == 0 else min(
                        mY, -(-(PAD + dyp_map[bi - 1][s] + 1) // 32) * 32)
                    # exclusive-from-left column start (segment s-1 already
                    # wrote the overlap [c0, cL))
                    cL = max(0, xlo) if s == 0 else max(
                        max(0, xlo), IL * SP * s + dxp_map[bi][s - 1] + 1)
                    # spill deferred two segments (see emit_spill)
                    pending.append((pseg, a, mY, xlo, XTs, v0, cL))
                if idx == 1 and pending_norm is not None:
                    norm_blk(pending_norm)
                    pending_norm = None
            # all spills into storage block `blk` are issued within the first
            # band of block blk+1 (deque depth 2); its normalize is deferred
            # there too so the flush never stalls the DVE queue head
            pending_norm = blk
        for st in pending:
            emit_spill(st)
        norm_blk(NBLK - 1, last=True)

    nc.compile()
    return nc


_PROG_CACHE = {}


def _get_program(disp, dt, dyp_map, dxn_map, dxp_map, SP, XT3, H=H, W=W):
    key = (disp, float(dt), tuple(tuple(r) for r in dyp_map),
           tuple(tuple(r) for r in dxn_map), tuple(tuple(r) for r in dxp_map),
           SP, XT3, H, W, MUL_ON_GPSIMD, ADD_ON_GPSIMD, PREP_ON_GPSIMD)
    if key not in _PROG_CACHE:
        _PROG_CACHE[key] = _build_program(disp, dt, dyp_map, dxn_map, dxp_map, SP, XT3, H=H, W=W)
    return _PROG_CACHE[key]


def _odd(v):
    return v if v % 2 == 1 else v + 1


def _window_params(fy, fx, dt, H=H, W=W):
    """Exact per-region displacement bounds (over all batch elements).
    Returns (disp, dyp_map, dxn_map, dxp_map, SP, XT3): disp = global |dy| max
    (top slack, keeps spills 32-aligned); per-(band,seg) bottom-y max and
    left/right x maxima (x maxima rounded up to odd so every spill/mul
    access stays 4-byte aligned for the DVE 2x mode); XT3 = psum extent."""
    dy = (dt * fy).max(axis=0)                     # [H, W] signed maxima
    dyn_ = (-(dt * fy)).max(axis=0)
    dxp_ = (dt * fx).max(axis=0)
    dxn_ = (-(dt * fx)).max(axis=0)
    disp = max(2, int(math.ceil(float(np.maximum(dy, dyn_).max()))))
    dxn_g = _odd(max(1, int(math.ceil(float(dxn_.max())))))
    dxp_g = _odd(max(1, int(math.ceil(float(dxp_.max())))))
    bands = [(a, min(BH, H - a)) for a in range(0, H, BH)]
    npairs = W // IL

    def build_maps(SPv):
        nsegs = (npairs + SPv - 1) // SPv
        dyp_map, dxn_map, dxp_map = [], [], []
        xt3 = 0
        for (a, bh) in bands:
            ry, rn, rp = [], [], []
            for s in range(nsegs):
                c0 = IL * SPv * s
                c1 = min(W, IL * SPv * (s + 1))
                ry.append(max(1, int(math.ceil(float(dy[a:a + bh, c0:c1].max())))))
                rn.append(min(dxn_g, _odd(max(1, int(math.ceil(float(dxn_[a:a + bh, c0:c1].max())))))))
                rp.append(min(dxp_g, _odd(max(1, int(math.ceil(float(dxp_[a:a + bh, c0:c1].max())))))))
                xt3 = max(xt3, (IL * ((c1 - c0) // IL) + rn[-1] + rp[-1] + 2) * 3)
            dyp_map.append(ry)
            dxn_map.append(rn)
            dxp_map.append(rp)
        return dyp_map, dxn_map, dxp_map, xt3

    # prefer SP=32 (5 uniform segments at W=640); psum is sized by the
    # regional window maxima, not the global bound, so this usually fits
    for SPv in (32,):
        dyp_map, dxn_map, dxp_map, xt3 = build_maps(SPv)
        if xt3 <= 512:
            return disp, dyp_map, dxn_map, dxp_map, SPv, xt3
    # fallback: global-bound sizing (always fits by construction)
    SPv = min(64, (512 // 3 - dxn_g - dxp_g - 2) // IL)
    dyp_map, dxn_map, dxp_map, xt3 = build_maps(SPv)
    return disp, dyp_map, dxn_map, dxp_map, SPv, min(512, xt3)


def kernel(flow_maps_x, flow_maps_y, i=0, tref=4):
    i = int(i)
    tref = int(tref)
    dt = float(tref - i)
    B = flow_maps_x.shape[0]
    assert B <= NCORES, f"batch {B} > {NCORES} cores not supported"
    fx = np.ascontiguousarray(flow_maps_x[:, i]).astype(np.float32)
    fy = np.ascontiguousarray(flow_maps_y[:, i]).astype(np.float32)

    disp, dyp_map, dxn_map, dxp_map, SPv, xt3 = _window_params(fy, fx, dt)
    nc = _get_program(disp, dt, dyp_map, dxn_map, dxp_map, SPv, xt3)
    in_maps = [{"fy": fy[b], "fx": fx[b]} for b in range(B)]
    res = run_bass_kernel_spmd(nc, in_maps, list(range(B)))
    wfx = np.stack([res.results[b]["out_wfx"] for b in range(B)])[:, None]
    wfy = np.stack([res.results[b]["out_wfy"] for b in range(B)])[:, None]
    return wfx.astype(np.float32), wfy.astype(np.float32)


def _ensure_ntff_hook():
    """The agent image lacks antenv.axon_hooks; synthesize it from trn_agent_boot."""
    import types
    try:
        import antenv.axon_hooks  # noqa: F401
        return
    except ImportError:
        pass
    from trn_agent_boot.trn_boot import _ntff_profile_via_ctypes
    hook = _ntff_profile_via_ctypes("/opt/axon/libaxon_pjrt.so")
    m = types.ModuleType("antenv.axon_hooks")
    m.get_axon_ntff_profile_hook = lambda: hook
    m.set_axon_ntff_profile_hook = lambda h: None
    sys.modules["antenv.axon_hooks"] = m


def timed_run(np_inputs):
    """Run once with NTFF tracing; return HW exec time in ns (max over traced cores)."""
    _ensure_ntff_hook()
    i = int(np_inputs["i"]); tref = int(np_inputs["tref"])
    dt = float(tref - i)
    fx = np.ascontiguousarray(np_inputs["flow_maps_x"][:, i]).astype(np.float32)
    fy = np.ascontiguousarray(np_inputs["flow_maps_y"][:, i]).astype(np.float32)
    B = fx.shape[0]
    disp, dyp_map, dxn_map, dxp_map, SPv, xt3 = _window_params(fy, fx, dt)
    nc = _get_program(disp, dt, dyp_map, dxn_map, dxp_map, SPv, xt3)
    in_maps = [{"fy": fy[b], "fx": fx[b]} for b in range(B)]
    res = run_bass_kernel_spmd(nc, in_maps, list(range(B)), trace=True)
    return res.exec_time_ns


if __name__ == "__main__":
    rng = np.random.default_rng(0)
    fmx = rng.standard_normal((8, 4, H, W), dtype=np.float32)
    fmy = rng.standard_normal((8, 4, H, W), dtype=np.float32)
    ox, oy = kernel(fmx, fmy, 0, 4)
    print(ox.shape, oy.shape, ox.dtype)
